# revision 46
# baseline (speedup 1.0000x reference)
"""CSC-TV primal-dual solver on 8 Trainium2 NeuronCores (v2, bf16 PE path).

Sharding: mb(2) x p-groups(4 of 8 filters) = 8 cores. Each core holds one
batch sample's full x/y1/y2 state (replicated within its mb-group of 4
cores) plus an 8-filter shard of `a`.

v2 design:
- All matmuls bf16 (states that feed the PE kept as bf16; x/y1 and DVE
  accumulators stay f32). N=512 matmuls via 3D access patterns.
- The AllReduce payload is (Ba_part - x/4) in bf16, so the reduced
  result is -(x - Ba) = -r directly: censored shift matmuls add -x/4
  into the conv PSUM windows. convT windows then load straight from the
  bounce buffer with 3 DMAs/channel; W2 is sign-flipped to absorb -r.
- One AllReduce per channel, pipelined against conv/convT of the other
  channels.
- Separable (rank-1) blur: banded y-pass matmul + corner matmuls from
  32-aligned windows, then 5 diagonal x-shift matmuls.
- Soft-threshold via two scalar-engine Relu passes; sqrt on scalar;
  reciprocal via the fast DVE approximation.
"""
import numpy as np
import ml_dtypes

EPS = 1e-8
ALPHA = 0.05
KS = 10          # iterations
C = 3
IM = 256         # image side
K7 = 7
FK = 5
PL = 8           # filters per core
X = 262          # 256 + 2*3 x-margins
NT = 8           # 32-row y tiles
WW = 38          # conv out-window rows
NSC = 10         # scalar slots per step
EPS_R = float(ALPHA * np.sqrt(np.float32(C * IM * IM)))

_NC_CACHE = {}
_LAST_IN_MAPS = None
_PHASES = []

# ---------------------------------------------------------------- walrus fixes


def _apply_walrus_workarounds():
    import concourse.tile as tile
    from concourse.vector_clock import ScopedClock, VectorClock

    def _chunked_drain_and_barrier(self, tick_clock, wait_clock):
        vec = list(tick_clock.global_clock)
        for i, tick in enumerate(vec):
            if tick <= 0:
                continue
            sub = [0] * len(vec)
            sub[i] = tick
            drain_inst = self.nc.sync.drain()
            wait_clock.add_sem_waits(
                drain_inst.ins, ScopedClock({None: VectorClock(sub)}))
        self.nc.all_engine_barrier()
        assert self.sems is not None
        popped = self.nc._tile_sem_poison_stack.pop()
        assert popped is self._sem_poison
        self.nc.clear_and_free_semaphores(
            list(self.sems.allocated().values()))
        self.nc.all_engine_barrier()

    tile.TileContext._drain_and_barrier = _chunked_drain_and_barrier


def _split_sync_waits(nc):
    """This walrus build allows a single sync-wait command per
    instruction; hoist extras onto same-engine no-ops."""
    from concourse import mybir
    for fn in nc.m.functions:
        for bb in fn.blocks:
            out = []
            for ins in bb.instructions:
                si = ins.sync_info
                if si is not None and si.on_wait and len(si.on_wait) > 1:
                    waits = list(si.on_wait)
                    extra, keep = waits[:-1], waits[-1:]
                    for k, w in enumerate(extra):
                        out.append(mybir.InstNoOp(
                            name=f"{ins.name}-ws{k}",
                            sync_info=mybir.SyncInfo(
                                on_wait=[w], on_update=[]),
                            bass_nofuse=True,
                            engine=ins.engine))
                    ins.sync_info = mybir.SyncInfo(
                        on_wait=keep, on_update=list(si.on_update))
                out.append(ins)
            try:
                bb.instructions = out
            except Exception:
                bb.instructions.clear()
                for i in out:
                    bb.instructions.append(i)


# ---------------------------------------------------------------- band builders

BF = ml_dtypes.bfloat16


def _build_w1(Bsh):
    # Bsh: (PL, C, 7, 7). W1[c,g,dx]: [128=(p4,y32), 38]
    W1 = np.zeros((C, 2, K7, 128, WW), np.float32)
    yi = np.arange(32)[:, None]
    m = np.arange(WW)[None, :]
    dy = yi - m + 6                       # [32, WW]
    msk = (dy >= 0) & (dy < 7)
    dyc = np.clip(dy, 0, 6)
    for c in range(C):
        for g in range(2):
            for dx in range(K7):
                for p in range(4):
                    vals = Bsh[4 * g + p, c, dyc, dx] * msk
                    W1[c, g, dx, 32 * p:32 * p + 32, :] = vals
    return W1.astype(BF)


def _build_w2n(Bsh):
    # W2n[c,g,dx]: [38, 128], sign-flipped adjoint band
    W2 = np.zeros((C, 2, K7, WW, 128), np.float32)
    yi = np.arange(32)[None, :]
    k = np.arange(WW)[:, None]
    dy = k - yi                           # [WW, 32]
    msk = (dy >= 0) & (dy < 7)
    dyc = np.clip(dy, 0, 6)
    for c in range(C):
        for g in range(2):
            for dx in range(K7):
                for p in range(4):
                    vals = Bsh[4 * g + p, c, 6 - dyc, 6 - dx] * msk
                    W2[c, g, dx, :, 32 * p:32 * p + 32] = vals
    return (-W2).astype(BF)


def _build_sep_blur(fil):
    # fil rank-1: fil = outer(gc, gr)
    u, s, vt = np.linalg.svd(fil.astype(np.float64))
    gc = (u[:, 0] * np.sqrt(s[0]))
    gr = (vt[0] * np.sqrt(s[0]))
    if gc[FK // 2] < 0:
        gc, gr = -gc, -gr
    gc = gc.astype(np.float32)
    gr = gr.astype(np.float32)
    # y-pass band: out[m] = sum_d gc[d] * in[m+d-2]
    Wy = np.zeros((128, 128), np.float32)
    kk = np.arange(128)[:, None]
    m = np.arange(128)[None, :]
    d = kk - m + 2
    msk = (d >= 0) & (d < FK)
    Wy[msk] = gc[np.clip(d, 0, FK - 1)][msk]
    # corner up: moving = full next block; rows 0,1 feed this block out 126,127
    Cu = np.zeros((128, 128), np.float32)
    for p in range(128):
        for mm in range(128):
            dd = 128 + p - mm + 2
            if 0 <= dd < FK:
                Cu[p, mm] = gc[dd]
    # corner down: moving = full prev block; rows 126,127 feed next out 0,1
    Cd = np.zeros((128, 128), np.float32)
    for p in range(128):
        for mm in range(128):
            dd = p - (128 + mm) + 2
            if 0 <= dd < FK:
                Cd[p, mm] = gc[dd]
    # x-pass diagonals
    Dg = np.zeros((FK, 128, 128), np.float32)
    for dx in range(FK):
        Dg[dx] = np.eye(128, dtype=np.float32) * gr[dx]
    return Wy.astype(BF), Cu.astype(BF), Cd.astype(BF), Dg.astype(BF)


def _build_shift_s():
    # Window t places -0.25*x at window-row m (= image row 32t-3+m) only
    # for the rows it "owns": m>=6 (m>=3 for t=0), so stitched overlaps
    # get the x-term exactly once. p = 32j-3+m into the moving x block.
    # Variants: 0: t=0 (j=0, m>=3); 1: t=4 (j=0, m>=6); 2: j=1; 3: j=2;
    # 4: j=3 (m>=6; m>=35 spills to the Ct corner, emitted for t=3 only).
    S = np.zeros((5, 128, WW), np.float32)
    specs = [(0, 3), (0, 6), (1, 6), (2, 6), (3, 6)]
    for v, (j, mlo) in enumerate(specs):
        for m in range(mlo, WW):
            p = 32 * j - 3 + m
            if 0 <= p < 128:
                S[v, p, m] = -0.25
    # tail corner (window t=3): image rows 128..130 = block h1 parts 0..2
    Ct = np.zeros((128, WW), np.float32)
    for p in range(3):
        Ct[p, 35 + p] = -0.25
    return S.astype(BF), Ct.astype(BF)


def _build_shift_w():
    # Window-extraction matrices: window t rows m <- block partition
    # p = 32(t%4)-3+m. Variants 0..3 = t%4; 4 = t=3 tail (h1 rows 0..2);
    # 5 = t=4 head (h0 rows 125..127).
    Sh = np.zeros((6, 128, WW), np.float32)
    for j in range(4):
        for m in range(WW):
            p = 32 * j - 3 + m
            if 0 <= p < 128:
                Sh[j, p, m] = 1.0
    for p in range(3):
        Sh[4, p, 35 + p] = 1.0
    for p in range(125, 128):
        Sh[5, p, p - 125] = 1.0
    return Sh.astype(BF)


def _build_sc(lam1, lam2, gam1, gam2, gam3):
    sc = np.zeros((1, 128), np.float32)
    for k in range(KS):
        g1 = np.float32(gam1[k])
        g2 = np.float32(gam2[k])
        g3 = np.float32(gam3[k])
        l1, l2 = np.float32(lam1[k]), np.float32(lam2[k])
        g3e = np.float32(g3 + np.float32(EPS))
        o = k * NSC
        sc[0, o + 0] = -g1
        sc[0, o + 1] = g2
        sc[0, o + 2] = -(g2 * l1)
        sc[0, o + 3] = (np.float32(EPS) * g3e) ** 2
        sc[0, o + 4] = g3
        sc[0, o + 5] = -l2
        sc[0, o + 6] = -(g3 / g3e)
        sc[0, o + 7] = np.float32(1.0) / g3e
        sc[0, o + 8] = -g3
        sc[0, o + 9] = np.float32(EPS) * g3e
    return sc


# ---------------------------------------------------------------- device build


def _build_nc():
    import concourse.bass as bass
    import concourse.mybir as mybir
    import concourse.tile as tile

    _apply_walrus_workarounds()

    F32 = mybir.dt.float32
    BF16 = mybir.dt.bfloat16
    AX = mybir.AluOpType
    AF = mybir.ActivationFunctionType
    AXL = mybir.AxisListType

    nc = bass.Bass()
    a_in = nc.dram_tensor("a_in", [PL, C, IM, IM], BF16, kind="ExternalInput")
    zr_in = nc.dram_tensor("zr_in", [C, IM, IM], BF16, kind="ExternalInput")
    zf_in = nc.dram_tensor("zf_in", [C, IM, IM], F32, kind="ExternalInput")
    w1_in = nc.dram_tensor("w1_in", [C, 2, K7, 128, WW], BF16,
                           kind="ExternalInput")
    w2_in = nc.dram_tensor("w2_in", [C, 2, K7, WW, 128], BF16,
                           kind="ExternalInput")
    wy_in = nc.dram_tensor("wy_in", [128, 128], BF16, kind="ExternalInput")
    cu_in = nc.dram_tensor("cu_in", [128, 128], BF16, kind="ExternalInput")
    cd_in = nc.dram_tensor("cd_in", [128, 128], BF16, kind="ExternalInput")
    dg_in = nc.dram_tensor("dg_in", [FK, 128, 128], BF16, kind="ExternalInput")
    s_in = nc.dram_tensor("s_in", [5, 128, WW], BF16, kind="ExternalInput")
    ct_in = nc.dram_tensor("ct_in", [128, WW], BF16, kind="ExternalInput")
    shw_in = nc.dram_tensor("shw_in", [6, 128, WW], BF16,
                            kind="ExternalInput")
    sc_in = nc.dram_tensor("sc_in", [1, 128], F32, kind="ExternalInput")
    on1_in = nc.dram_tensor("on1_in", [1, 128], F32, kind="ExternalInput")
    on128_in = nc.dram_tensor("on128_in", [128, 128], F32,
                              kind="ExternalInput")
    x_out = nc.dram_tensor("x_out", [C, IM, IM], F32, kind="ExternalOutput")

    RG = [[0, 1, 2, 3], [4, 5, 6, 7]]

    def mark(label):
        _PHASES.append((label, int(nc.get_next_instruction_name()[2:])))

    with tile.TileContext(nc) as tc:
        with (
            tc.tile_pool(name="const", bufs=1) as cpool,
            tc.tile_pool(name="state", bufs=1) as spool,
            tc.tile_pool(name="tmp", bufs=2) as tpool,
            tc.tile_pool(name="stg", bufs=2) as stgpool,
            tc.tile_pool(name="pwin", bufs=3, space="PSUM") as pwin,
            tc.tile_pool(name="pda", bufs=2, space="PSUM") as pda,
            tc.tile_pool(name="pby", bufs=1, space="PSUM") as pby,
            tc.tile_pool(name="pbx", bufs=1, space="PSUM") as pbx,
            tc.tile_pool(name="pmisc", bufs=1, space="PSUM") as pmisc,
            tc.tile_pool(name="dram", bufs=1, space="DRAM") as dpool,
        ):
            # ---------------- constants
            W1, W2n = {}, {}
            for c in range(C):
                for g in range(2):
                    for dx in range(K7):
                        t1_ = cpool.tile([128, WW], BF16, tag=f"w1_{c}_{g}_{dx}")
                        nc.sync.dma_start(t1_[:], w1_in[c, g, dx])
                        W1[c, g, dx] = t1_
                        t2_ = cpool.tile([WW, 128], BF16, tag=f"w2_{c}_{g}_{dx}")
                        nc.sync.dma_start(t2_[:], w2_in[c, g, dx])
                        W2n[c, g, dx] = t2_
            Wy = cpool.tile([128, 128], BF16, tag="wy")
            nc.sync.dma_start(Wy[:], wy_in[:])
            Cu = cpool.tile([128, 128], BF16, tag="cu")
            nc.sync.dma_start(Cu[:], cu_in[:])
            Cd = cpool.tile([128, 128], BF16, tag="cd")
            nc.sync.dma_start(Cd[:], cd_in[:])
            Dg = {}
            for dx in range(FK):
                t_ = cpool.tile([128, 128], BF16, tag=f"dg_{dx}")
                nc.sync.dma_start(t_[:], dg_in[dx])
                Dg[dx] = t_
            Sm = {}
            for j in range(5):
                t_ = cpool.tile([128, WW], BF16, tag=f"sm_{j}")
                nc.sync.dma_start(t_[:], s_in[j])
                Sm[j] = t_
            SVAR = {0: 0, 4: 1, 1: 2, 5: 2, 2: 3, 6: 3, 3: 4, 7: 4}
            Ct = cpool.tile([128, WW], BF16, tag="ct")
            nc.sync.dma_start(Ct[:], ct_in[:])
            SH = {}
            for v in range(6):
                t_ = cpool.tile([128, WW], BF16, tag=f"shw_{v}")
                nc.sync.dma_start(t_[:], shw_in[v])
                SH[v] = t_
            ones1 = cpool.tile([1, 128], F32, tag="ones1")
            nc.sync.dma_start(ones1[:], on1_in[:])
            ones128 = cpool.tile([128, 128], F32, tag="ones128")
            nc.sync.dma_start(ones128[:], on128_in[:])
            sc_raw = cpool.tile([1, 128], F32, tag="sc_raw")
            nc.sync.dma_start(sc_raw[:], sc_in[:])
            sc_ps = pmisc.tile([128, 128], F32, tag="msc")
            nc.tensor.matmul(sc_ps[:], ones1[:], sc_raw[:],
                             start=True, stop=True)
            sc_b = cpool.tile([128, 128], F32, tag="sc_b")
            nc.vector.tensor_copy(sc_b[:], sc_ps[:])

            def scap(k, j):
                return sc_b[:, k * NSC + j:k * NSC + j + 1]

            # ---------------- state
            # a: bf16 margined [128=(4p,32y), 8 tiles x 262]
            A = {}
            for c in range(C):
                for g in range(2):
                    ta = spool.tile([128, NT * X], BF16, tag=f"a_{c}_{g}")
                    nc.vector.memset(ta[:], 0.0)
                    for p in range(4):
                        dst = ta[32 * p:32 * p + 32, :].rearrange(
                            "y (t x) -> y t x", x=X)[:, :, 3:259]
                        src = a_in[4 * g + p, c].rearrange(
                            "(t y) x -> y t x", y=32)
                        nc.sync.dma_start(dst, src)
                    A[c, g] = ta

            xA = spool.tile([128, 1536], F32, tag="xA")
            xB = spool.tile([128, 1536], F32, tag="xB")
            xbf = spool.tile([128, 1536], BF16, tag="xbf")
            y1a = spool.tile([128, 1536], F32, tag="y1a")
            y1b = spool.tile([128, 1536], F32, tag="y1b")
            nc.vector.memset(y1a[:], 0.0)
            nc.vector.memset(y1b[:], 0.0)
            y2m = spool.tile([128, 1536], BF16, tag="y2m")
            nc.vector.memset(y2m[:], 0.0)
            vm = spool.tile([128, 1536], BF16, tag="vm")
            zbf = spool.tile([128, 1536], BF16, tag="zbf")
            zt = spool.tile([128, 1536], F32, tag="zt")
            rn = spool.tile([128, 1536], BF16, tag="rn")
            rwin = spool.tile([WW, C * NT * X], BF16, tag="rwin")
            nc.vector.memset(rwin[:], 0.0)
            d0t = spool.tile([128, 1536], F32, tag="d0t")
            dvt = spool.tile([128, 1536], BF16, tag="dvt")
            dty = spool.tile([128, 1536], F32, tag="dty")
            tt1 = spool.tile([128, 1536], F32, tag="tt1")
            tt2 = spool.tile([128, 1536], F32, tag="tt2")

            src = zr_in[:].rearrange("c (h p) x -> p c h x", h=2)
            dst = zbf[:].rearrange("p (c h x) -> p c h x", c=C, h=2)
            nc.sync.dma_start(dst, src)
            src = zf_in[:].rearrange("c (h p) x -> p c h x", h=2)
            dst = zt[:].rearrange("p (c h x) -> p c h x", c=C, h=2)
            nc.sync.dma_start(dst, src)

            bnci, bnco = {}, {}
            for c in range(C):
                bnci[c] = dpool.tile([IM, IM], BF16, tag=f"bnci_{c}",
                                     name=f"bnci_{c}")
                bnco[c] = dpool.tile([IM, IM], BF16, tag=f"bnco_{c}",
                                     name=f"bnco_{c}")

            def cblk(t, c):
                return t[:, 512 * c:512 * c + 512]

            def chx(t, c, h):
                return t[:, 512 * c + 256 * h:512 * c + 256 * h + 256]

            def pair2(t, c):
                # [128, 2, 256] over the two h blocks of channel c
                return t[:, 512 * c:512 * c + 512].rearrange(
                    "p (h x) -> p h x", x=256)

            # blur of one channel: src bf16 [128,1536] -> consume(pxm)
            def blur_c(src, c, consume):
                py = pby.tile([128, 512], F32, tag="by")
                nc.tensor.matmul(py[:], Wy[:], pair2(src, c),
                                 start=True, stop=False)
                nc.tensor.matmul(py[:, 0:256], Cu[:],
                                 src[:, 512 * c + 256:512 * c + 512],
                                 start=False, stop=False)
                nc.tensor.matmul(py[:, 256:512], Cd[:],
                                 src[:, 512 * c:512 * c + 256],
                                 start=False, stop=True)
                pyb = tpool.tile([128, 520], BF16, tag="pyb")
                nc.vector.memset(pyb[:, 0:2], 0.0)
                nc.vector.memset(pyb[:, 258:262], 0.0)
                nc.vector.memset(pyb[:, 518:520], 0.0)
                pv = pyb[:].rearrange("p (h x) -> p h x", x=260)
                nc.scalar.copy(pv[:, :, 2:258],
                               py[:].rearrange("p (h x) -> p h x", x=256))
                pxm = pbx.tile([128, 512], F32, tag="bx")
                for dx in range(FK):
                    nc.tensor.matmul(
                        pxm[:], Dg[dx], pv[:, :, dx:dx + 256],
                        start=(dx == 0), stop=(dx == FK - 1))
                consume(pxm)

            # x0 = blur(z)
            for c in range(C):
                def init_consume(pxm, c=c):
                    nc.vector.tensor_copy(cblk(xA, c), pxm[:])
                    nc.scalar.copy(cblk(xbf, c), pxm[:])

                blur_c(zbf, c, init_consume)

            xcur, xnxt = xA, xB
            for k in range(KS):
                # ---- conv_CSC + (-x/4) + stage + AR, per channel
                for c in range(C):
                    mark(f'k{k}_conv{c}')
                    st = stgpool.tile([WW, NT * 256], BF16, tag="stage")

                    def stitch(t, pairs, st=st, c=c):
                        # window t: copy rows 0:32; rows 32:38 get the
                        # overlap-add with window t+1 (if any)
                        pi, half = t // 2, 256 * (t % 2)
                        nc.scalar.copy(
                            st[0:32, 256 * t:256 * t + 256],
                            pairs[pi][0:32, half:half + 256])
                        nc.scalar.copy(
                            st[32:38, 256 * t:256 * t + 256],
                            pairs[pi][32:38, half:half + 256])
                        if t < NT - 1:
                            pj, hj = (t + 1) // 2, 256 * ((t + 1) % 2)
                            nc.vector.tensor_add(
                                st[32:38, 256 * t:256 * t + 256],
                                st[32:38, 256 * t:256 * t + 256],
                                pairs[pj][0:6, hj:hj + 256])

                    pairs = {}
                    for pi in range(4):
                        pq = pwin.tile([WW, 512], F32, tag="baq")
                        first = True
                        for g in range(2):
                            av = A[c, g][:].rearrange(
                                "p (t x) -> p t x", x=X)
                            for dx in range(K7):
                                mv = av[:, 2 * pi:2 * pi + 2,
                                        dx:dx + 256]
                                nc.tensor.matmul(pq[:], W1[c, g, dx], mv,
                                                 start=first, stop=False)
                                first = False
                        for ti in range(2):
                            t = 2 * pi + ti
                            h = t // 4
                            col = 256 * ti
                            last = (ti == 1) and t != 3
                            nc.tensor.matmul(
                                pq[0:WW, col:col + 256], Sm[SVAR[t]],
                                xbf[:, 512 * c + 256 * h:
                                    512 * c + 256 * h + 256],
                                start=False, stop=last)
                            if t == 3:
                                nc.tensor.matmul(
                                    pq[0:WW, col:col + 256], Ct,
                                    xbf[:, 512 * c + 256:512 * c + 512],
                                    start=False, stop=True)
                        pairs[pi] = pq
                        if pi >= 1:
                            stitch(2 * pi - 2, pairs)
                            stitch(2 * pi - 1, pairs)
                            del pairs[pi - 1]
                    stitch(6, pairs)
                    stitch(7, pairs)
                    # 3 DMAs -> bnci[c]
                    dstA = bnci[c][3:227, :].rearrange(
                        "(q r) x -> r q x", r=32)
                    srcA = st[6:38, 0:7 * 256].rearrange(
                        "r (q x) -> r q x", x=256)
                    nc.sync.dma_start(dstA, srcA)
                    nc.sync.dma_start(bnci[c][227:256, :],
                                      st[6:35, 7 * 256:8 * 256])
                    nc.sync.dma_start(bnci[c][0:3, :], st[3:6, 0:256])
                    nc.gpsimd.collective_compute(
                        "AllReduce", AX.add, replica_groups=RG,
                        ins=[bnci[c].opt()], outs=[bnco[c].opt()])

                # ---- load -r as soon as each AR lands
                mark(f'k{k}_loads')
                for c in range(C):
                    nc.sync.dma_start(
                        cblk(rn, c).rearrange("p (h x) -> p h x", x=256),
                        bnco[c][:].rearrange("(h p) x -> p h x", h=2))

                # ---- window build (PE) + convT + a update
                for c in range(C):
                    mark(f'k{k}_convT{c}')
                    # extract the 8 windows of -r for this channel from rn
                    # via shift matmuls; scalar-copy psum -> rwin interior
                    rwv = rwin[:].rearrange("r (t x) -> r t x", x=X)
                    for pi in range(4):
                        pw = pwin.tile([WW, 512], F32, tag="baq")
                        for ti in range(2):
                            t = 2 * pi + ti
                            j, h = t % 4, t // 4
                            col = 256 * ti
                            nc.tensor.matmul(
                                pw[0:WW, col:col + 256], SH[j],
                                rn[:, 512 * c + 256 * h:
                                   512 * c + 256 * h + 256],
                                start=True, stop=(t not in (3, 4)))
                            if t == 3:
                                nc.tensor.matmul(
                                    pw[0:WW, col:col + 256], SH[4],
                                    rn[:, 512 * c + 256:512 * c + 512],
                                    start=False, stop=True)
                            if t == 4:
                                nc.tensor.matmul(
                                    pw[0:WW, col:col + 256], SH[5],
                                    rn[:, 512 * c:512 * c + 256],
                                    start=False, stop=True)
                        dstP = rwv[:, NT * c + 2 * pi:NT * c + 2 * pi + 2,
                                   3:259]
                        nc.scalar.copy(
                            dstP, pw[:].rearrange("r (b x) -> r b x", x=256))
                    for g in range(2):
                        for cc in range(2):
                            tav = tpool.tile([128, 1024], F32, tag="tav")
                            for pp in range(2):
                                p2 = 2 * cc + pp
                                pd = pda.tile([128, 512], F32, tag="da")
                                mv = rwv[:, NT * c + 2 * p2:
                                         NT * c + 2 * p2 + 2, :]
                                for i, dx in enumerate(range(K7)):
                                    nc.tensor.matmul(
                                        pd[:], W2n[c, g, dx],
                                        mv[:, :, dx:dx + 256],
                                        start=(i == 0), stop=(i == K7 - 1))
                                av = A[c, g][:].rearrange(
                                    "p (t x) -> p t x", x=X)[
                                    :, 2 * p2:2 * p2 + 2, 3:259]
                                nc.vector.scalar_tensor_tensor(
                                    tav[:, 512 * pp:512 * pp + 512].rearrange(
                                        "p (b x) -> p b x", x=256),
                                    pd[:].rearrange("p (b x) -> p b x", x=256),
                                    scap(k, 1), av,
                                    op0=AX.mult, op1=AX.add)
                            s1 = tpool.tile([128, 1024], F32, tag="s1")
                            s2 = tpool.tile([128, 1024], F32, tag="s2")
                            nc.scalar.activation(s1[:], tav[:], AF.Relu,
                                                 bias=scap(k, 2), scale=1.0)
                            nc.scalar.activation(s2[:], tav[:], AF.Relu,
                                                 bias=scap(k, 2), scale=-1.0)
                            av4 = A[c, g][:].rearrange(
                                "p (t x) -> p t x", x=X)[
                                :, 4 * cc:4 * cc + 4, 3:259]
                            nc.vector.tensor_sub(
                                av4,
                                s1[:].rearrange("p (t x) -> p t x", x=256),
                                s2[:].rearrange("p (t x) -> p t x", x=256))

                # ---- deferred prox of the previous step (overlaps the
                # conv/convT PE work of this step)
                if k >= 1:
                    mark(f'k{k}_prox')
                    kp = k - 1
                    nc.scalar.square(tt1[:], y1a[:])
                    nc.scalar.square(tt2[:], y1b[:])
                    nc.vector.tensor_add(tt1[:], tt1[:], tt2[:])
                    nc.scalar.activation(tt2[:], tt1[:], AF.Ln,
                                         bias=scap(kp, 3), scale=1.0)
                    nc.scalar.activation(tt1[:], tt2[:], AF.Exp,
                                         bias=0.0, scale=-0.5)
                    nc.scalar.activation(tt2[:], tt1[:], AF.Relu,
                                         bias=1.0, scale=scap(kp, 5))
                    nc.scalar.activation(tt1[:], tt2[:], AF.Identity,
                                         bias=1.0, scale=scap(kp, 6))
                    nc.vector.tensor_mul(y1a[:], y1a[:], tt1[:])
                    nc.vector.tensor_mul(y1b[:], y1b[:], tt1[:])
                    nc.vector.scalar_tensor_tensor(
                        tt1[:], y2m[:], scap(kp, 7), zt[:],
                        op0=AX.mult, op1=AX.subtract)
                    nc.scalar.square(tt2[:], tt1[:])
                    nrs = tpool.tile([128, 1], F32, tag="nrs")
                    nc.vector.tensor_reduce(nrs[:], tt2[:], axis=AXL.X,
                                            op=AX.add)
                    tot = pmisc.tile([128, 128], F32, tag="msc")
                    nc.tensor.matmul(tot[:, 0:1], ones128[:], nrs[:],
                                     start=True, stop=True)
                    fsc = tpool.tile([128, 1], F32, tag="fsc")
                    nc.scalar.sqrt(fsc[:], tot[:, 0:1])
                    nc.vector.tensor_scalar_max(fsc[:], fsc[:], float(EPS))
                    rc2 = tpool.tile([128, 1], F32, tag="rc2")
                    nc.vector.reciprocal(rc2[:], fsc[:])
                    nc.vector.tensor_scalar(fsc[:], rc2[:], EPS_R, 1.0,
                                            op0=AX.mult, op1=AX.min)
                    nc.vector.scalar_tensor_tensor(
                        tt2[:], tt1[:], fsc[:], zt[:],
                        op0=AX.mult, op1=AX.add)
                    for c in range(C):
                        nc.vector.scalar_tensor_tensor(
                            cblk(y2m, c), cblk(tt2, c), scap(kp, 8),
                            cblk(y2m, c), op0=AX.mult, op1=AX.add)

                # ---- dty = Dt(y1) + blur(y2) - r, then x update + v
                mark(f'k{k}_dty')
                nc.sync.dma_start(d0t[1:128, :], y1a[0:127, :])
                for h in range(2):
                    dsh = d0t[0:1, :].rearrange(
                        "p (c h x) -> p c h x", c=C, h=2)[:, :, h:h + 1, :]
                    ssh = y1a[127:128, :].rearrange(
                        "p (c h x) -> p c h x", c=C, h=2)[:, :, 1 - h:2 - h, :]
                    nc.sync.dma_start(dsh, ssh)
                nc.vector.tensor_sub(dty[:], d0t[:], y1a[:])
                v6o = tt1[:].rearrange("p (b x) -> p b x", x=256)
                v6i = y1b[:].rearrange("p (b x) -> p b x", x=256)
                nc.vector.tensor_sub(v6o[:, :, 1:256], v6i[:, :, 0:255],
                                     v6i[:, :, 1:256])
                nc.vector.tensor_sub(v6o[:, :, 0:1], v6i[:, :, 255:256],
                                     v6i[:, :, 0:1])
                nc.vector.tensor_add(dty[:], dty[:], tt1[:])

                for c in range(C):
                    def y2blur_consume(pxm, c=c):
                        nc.vector.tensor_add(cblk(dty, c), cblk(dty, c),
                                             pxm[:])

                    blur_c(y2m, c, y2blur_consume)
                    nc.vector.tensor_sub(cblk(dty, c), cblk(dty, c),
                                         cblk(rn, c))
                    nc.vector.scalar_tensor_tensor(
                        cblk(tt2, c), cblk(dty, c), scap(k, 0), cblk(xcur, c),
                        op0=AX.mult, op1=AX.add)
                    nc.vector.tensor_scalar(cblk(xnxt, c), cblk(tt2, c),
                                            0.0, 1.0, op0=AX.max, op1=AX.min)
                    nc.vector.scalar_tensor_tensor(
                        cblk(vm, c), cblk(xnxt, c), 2.0, cblk(xcur, c),
                        op0=AX.mult, op1=AX.subtract)
                    nc.scalar.copy(cblk(xbf, c), cblk(xnxt, c))

                    def y2_consume(pxm, c=c, k=k):
                        nc.vector.scalar_tensor_tensor(
                            cblk(y2m, c), pxm[:], scap(k, 4), cblk(y2m, c),
                            op0=AX.mult, op1=AX.add)

                    blur_c(vm, c, y2_consume)

                # ---- y1/y2 accumulate (prox deferred to next iteration)
                mark(f'k{k}_acc')
                nc.sync.dma_start(dvt[0:127, :], vm[1:128, :])
                for h in range(2):
                    dsh = dvt[127:128, :].rearrange(
                        "p (c h x) -> p c h x", c=C, h=2)[:, :, h:h + 1, :]
                    ssh = vm[0:1, :].rearrange(
                        "p (c h x) -> p c h x", c=C, h=2)[:, :, 1 - h:2 - h, :]
                    nc.sync.dma_start(dsh, ssh)
                nc.vector.tensor_sub(tt1[:], dvt[:], vm[:])
                nc.vector.scalar_tensor_tensor(
                    y1a[:], tt1[:], scap(k, 4), y1a[:],
                    op0=AX.mult, op1=AX.add)
                v6o = tt2[:].rearrange("p (b x) -> p b x", x=256)
                v6i = vm[:].rearrange("p (b x) -> p b x", x=256)
                nc.vector.tensor_sub(v6o[:, :, 0:255], v6i[:, :, 1:256],
                                     v6i[:, :, 0:255])
                nc.vector.tensor_sub(v6o[:, :, 255:256], v6i[:, :, 0:1],
                                     v6i[:, :, 255:256])
                nc.vector.scalar_tensor_tensor(
                    y1b[:], tt2[:], scap(k, 4), y1b[:],
                    op0=AX.mult, op1=AX.add)

                xcur, xnxt = xnxt, xcur

            # ---------------- output
            dstO = x_out[:].rearrange("c (h p) x -> p c h x", h=2)
            srcO = xcur[:].rearrange("p (c h x) -> p c h x", c=C, h=2)
            nc.sync.dma_start(dstO, srcO)

    _split_sync_waits(nc)
    return nc


# ---------------------------------------------------------------- host entry


def kernel(z, a_init, B, fil, lam1, lam2, gam1, gam2, gam3):
    from concourse.bass_utils import run_bass_kernel_spmd

    if "nc" not in _NC_CACHE:
        _NC_CACHE["nc"] = _build_nc()
    nc = _NC_CACHE["nc"]

    z = np.asarray(z, np.float32)
    a_init = np.asarray(a_init, np.float32)
    B = np.asarray(B, np.float32)
    fil = np.asarray(fil, np.float32)
    sc = _build_sc(np.asarray(lam1, np.float32), np.asarray(lam2, np.float32),
                   np.asarray(gam1, np.float32), np.asarray(gam2, np.float32),
                   np.asarray(gam3, np.float32))
    Wy, Cu, Cd, Dg = _build_sep_blur(fil)
    Sm, Ct = _build_shift_s()
    Shw = _build_shift_w()
    on1 = np.ones((1, 128), np.float32)
    on128 = np.ones((128, 128), np.float32)

    in_maps = []
    for core in range(8):
        b, pg = core // 4, core % 4
        Bsh = B[PL * pg:PL * pg + PL]
        in_maps.append({
            "a_in": np.ascontiguousarray(
                a_init[b, PL * pg:PL * pg + PL]).astype(BF),
            "zr_in": np.ascontiguousarray(z[b]).astype(BF),
            "zf_in": np.ascontiguousarray(z[b]),
            "w1_in": _build_w1(Bsh),
            "w2_in": _build_w2n(Bsh),
            "wy_in": Wy, "cu_in": Cu, "cd_in": Cd, "dg_in": Dg,
            "s_in": Sm, "ct_in": Ct, "shw_in": Shw,
            "sc_in": sc, "on1_in": on1, "on128_in": on128,
        })
    global _LAST_IN_MAPS
    _LAST_IN_MAPS = in_maps
    res = run_bass_kernel_spmd(nc, in_maps, core_ids=list(range(8)))
    out = np.stack([res.results[0]["x_out"], res.results[4]["x_out"]])
    return out.astype(np.float32)


# revision 53
# speedup vs baseline: 1.1052x; 1.1052x over previous
"""CSC-TV primal-dual solver on 8 Trainium2 NeuronCores (v2, bf16 PE path).

Sharding: mb(2) x p-groups(4 of 8 filters) = 8 cores. Each core holds one
batch sample's full x/y1/y2 state (replicated within its mb-group of 4
cores) plus an 8-filter shard of `a`.

v2 design:
- All matmuls bf16 (states that feed the PE kept as bf16; x/y1 and DVE
  accumulators stay f32). N=512 matmuls via 3D access patterns.
- The AllReduce payload is (Ba_part - x/4) in bf16, so the reduced
  result is -(x - Ba) = -r directly: censored shift matmuls add -x/4
  into the conv PSUM windows. convT windows then load straight from the
  bounce buffer with 3 DMAs/channel; W2 is sign-flipped to absorb -r.
- One AllReduce per channel, pipelined against conv/convT of the other
  channels.
- Separable (rank-1) blur: banded y-pass matmul + corner matmuls from
  32-aligned windows, then 5 diagonal x-shift matmuls.
- Soft-threshold via two scalar-engine Relu passes; sqrt on scalar;
  reciprocal via the fast DVE approximation.
"""
import numpy as np
import ml_dtypes

EPS = 1e-8
ALPHA = 0.05
KS = 10          # iterations
C = 3
IM = 256         # image side
K7 = 7
FK = 5
PL = 8           # filters per core
X = 262          # 256 + 2*3 x-margins
NT = 8           # 32-row y tiles
WW = 38          # conv out-window rows
NSC = 10         # scalar slots per step
EPS_R = float(ALPHA * np.sqrt(np.float32(C * IM * IM)))

_NC_CACHE = {}
_LAST_IN_MAPS = None
_PHASES = []

# ---------------------------------------------------------------- walrus fixes


def _apply_walrus_workarounds():
    import concourse.tile as tile
    from concourse.vector_clock import ScopedClock, VectorClock

    def _chunked_drain_and_barrier(self, tick_clock, wait_clock):
        vec = list(tick_clock.global_clock)
        for i, tick in enumerate(vec):
            if tick <= 0:
                continue
            sub = [0] * len(vec)
            sub[i] = tick
            drain_inst = self.nc.sync.drain()
            wait_clock.add_sem_waits(
                drain_inst.ins, ScopedClock({None: VectorClock(sub)}))
        self.nc.all_engine_barrier()
        assert self.sems is not None
        popped = self.nc._tile_sem_poison_stack.pop()
        assert popped is self._sem_poison
        self.nc.clear_and_free_semaphores(
            list(self.sems.allocated().values()))
        self.nc.all_engine_barrier()

    tile.TileContext._drain_and_barrier = _chunked_drain_and_barrier


def _split_sync_waits(nc):
    """This walrus build allows a single sync-wait command per
    instruction; hoist extras onto same-engine no-ops."""
    from concourse import mybir
    for fn in nc.m.functions:
        for bb in fn.blocks:
            out = []
            for ins in bb.instructions:
                si = ins.sync_info
                if si is not None and si.on_wait and len(si.on_wait) > 1:
                    waits = list(si.on_wait)
                    extra, keep = waits[:-1], waits[-1:]
                    for k, w in enumerate(extra):
                        out.append(mybir.InstNoOp(
                            name=f"{ins.name}-ws{k}",
                            sync_info=mybir.SyncInfo(
                                on_wait=[w], on_update=[]),
                            bass_nofuse=True,
                            engine=ins.engine))
                    ins.sync_info = mybir.SyncInfo(
                        on_wait=keep, on_update=list(si.on_update))
                out.append(ins)
            try:
                bb.instructions = out
            except Exception:
                bb.instructions.clear()
                for i in out:
                    bb.instructions.append(i)


# ---------------------------------------------------------------- band builders

BF = ml_dtypes.bfloat16


def _build_w1(Bsh):
    # Bsh: (PL, C, 7, 7). W1[c,g,dx]: [128=(p4,y32), 38]
    W1 = np.zeros((C, 2, K7, 128, WW), np.float32)
    yi = np.arange(32)[:, None]
    m = np.arange(WW)[None, :]
    dy = yi - m + 6                       # [32, WW]
    msk = (dy >= 0) & (dy < 7)
    dyc = np.clip(dy, 0, 6)
    for c in range(C):
        for g in range(2):
            for dx in range(K7):
                for p in range(4):
                    vals = Bsh[4 * g + p, c, dyc, dx] * msk
                    W1[c, g, dx, 32 * p:32 * p + 32, :] = vals
    return W1.astype(BF)


def _build_w2n(Bsh):
    # W2n[c,g,dx]: [38, 128], sign-flipped adjoint band
    W2 = np.zeros((C, 2, K7, WW, 128), np.float32)
    yi = np.arange(32)[None, :]
    k = np.arange(WW)[:, None]
    dy = k - yi                           # [WW, 32]
    msk = (dy >= 0) & (dy < 7)
    dyc = np.clip(dy, 0, 6)
    for c in range(C):
        for g in range(2):
            for dx in range(K7):
                for p in range(4):
                    vals = Bsh[4 * g + p, c, 6 - dyc, 6 - dx] * msk
                    W2[c, g, dx, :, 32 * p:32 * p + 32] = vals
    return (-W2).astype(BF)


def _build_sep_blur(fil):
    # fil rank-1: fil = outer(gc, gr)
    u, s, vt = np.linalg.svd(fil.astype(np.float64))
    gc = (u[:, 0] * np.sqrt(s[0]))
    gr = (vt[0] * np.sqrt(s[0]))
    if gc[FK // 2] < 0:
        gc, gr = -gc, -gr
    gc = gc.astype(np.float32)
    gr = gr.astype(np.float32)
    # y-pass band: out[m] = sum_d gc[d] * in[m+d-2]
    Wy = np.zeros((128, 128), np.float32)
    kk = np.arange(128)[:, None]
    m = np.arange(128)[None, :]
    d = kk - m + 2
    msk = (d >= 0) & (d < FK)
    Wy[msk] = gc[np.clip(d, 0, FK - 1)][msk]
    # corner up: moving = full next block; rows 0,1 feed this block out 126,127
    Cu = np.zeros((128, 128), np.float32)
    for p in range(128):
        for mm in range(128):
            dd = 128 + p - mm + 2
            if 0 <= dd < FK:
                Cu[p, mm] = gc[dd]
    # corner down: moving = full prev block; rows 126,127 feed next out 0,1
    Cd = np.zeros((128, 128), np.float32)
    for p in range(128):
        for mm in range(128):
            dd = p - (128 + mm) + 2
            if 0 <= dd < FK:
                Cd[p, mm] = gc[dd]
    # x-pass diagonals
    Dg = np.zeros((FK, 128, 128), np.float32)
    for dx in range(FK):
        Dg[dx] = np.eye(128, dtype=np.float32) * gr[dx]
    return Wy.astype(BF), Cu.astype(BF), Cd.astype(BF), Dg.astype(BF)


def _build_shift_s():
    # Window t places -0.25*x at window-row m (= image row 32t-3+m) only
    # for the rows it "owns": m>=6 (m>=3 for t=0), so stitched overlaps
    # get the x-term exactly once. p = 32j-3+m into the moving x block.
    # Variants: 0: t=0 (j=0, m>=3); 1: t=4 (j=0, m>=6); 2: j=1; 3: j=2;
    # 4: j=3 (m>=6; m>=35 spills to the Ct corner, emitted for t=3 only).
    S = np.zeros((5, 128, WW), np.float32)
    specs = [(0, 3), (0, 6), (1, 6), (2, 6), (3, 6)]
    for v, (j, mlo) in enumerate(specs):
        for m in range(mlo, WW):
            p = 32 * j - 3 + m
            if 0 <= p < 128:
                S[v, p, m] = -0.25
    # tail corner (window t=3): image rows 128..130 = block h1 parts 0..2
    Ct = np.zeros((128, WW), np.float32)
    for p in range(3):
        Ct[p, 35 + p] = -0.25
    return S.astype(BF), Ct.astype(BF)


def _build_shift_w():
    # Window-extraction matrices: window t rows m <- block partition
    # p = 32(t%4)-3+m. Variants 0..3 = t%4; 4 = t=3 tail (h1 rows 0..2);
    # 5 = t=4 head (h0 rows 125..127).
    Sh = np.zeros((6, 128, WW), np.float32)
    for j in range(4):
        for m in range(WW):
            p = 32 * j - 3 + m
            if 0 <= p < 128:
                Sh[j, p, m] = 1.0
    for p in range(3):
        Sh[4, p, 35 + p] = 1.0
    for p in range(125, 128):
        Sh[5, p, p - 125] = 1.0
    return Sh.astype(BF)


def _build_sc(lam1, lam2, gam1, gam2, gam3):
    sc = np.zeros((1, 128), np.float32)
    for k in range(KS):
        g1 = np.float32(gam1[k])
        g2 = np.float32(gam2[k])
        g3 = np.float32(gam3[k])
        l1, l2 = np.float32(lam1[k]), np.float32(lam2[k])
        g3e = np.float32(g3 + np.float32(EPS))
        o = k * NSC
        sc[0, o + 0] = -g1
        sc[0, o + 1] = g2
        sc[0, o + 2] = -(g2 * l1)
        sc[0, o + 3] = (np.float32(EPS) * g3e) ** 2
        sc[0, o + 4] = g3
        sc[0, o + 5] = -l2
        sc[0, o + 6] = -(g3 / g3e)
        sc[0, o + 7] = np.float32(1.0) / g3e
        sc[0, o + 8] = -g3
        sc[0, o + 9] = np.float32(EPS) * g3e
    return sc


# ---------------------------------------------------------------- device build


def _build_nc():
    import concourse.bass as bass
    import concourse.mybir as mybir
    import concourse.tile as tile

    _apply_walrus_workarounds()

    F32 = mybir.dt.float32
    BF16 = mybir.dt.bfloat16
    AX = mybir.AluOpType
    AF = mybir.ActivationFunctionType
    AXL = mybir.AxisListType

    nc = bass.Bass()
    a_in = nc.dram_tensor("a_in", [PL, C, IM, IM], BF16, kind="ExternalInput")
    zr_in = nc.dram_tensor("zr_in", [C, IM, IM], BF16, kind="ExternalInput")
    zf_in = nc.dram_tensor("zf_in", [C, IM, IM], F32, kind="ExternalInput")
    w1_in = nc.dram_tensor("w1_in", [C, 2, K7, 128, WW], BF16,
                           kind="ExternalInput")
    w2_in = nc.dram_tensor("w2_in", [C, 2, K7, WW, 128], BF16,
                           kind="ExternalInput")
    wy_in = nc.dram_tensor("wy_in", [128, 128], BF16, kind="ExternalInput")
    cu_in = nc.dram_tensor("cu_in", [128, 128], BF16, kind="ExternalInput")
    cd_in = nc.dram_tensor("cd_in", [128, 128], BF16, kind="ExternalInput")
    dg_in = nc.dram_tensor("dg_in", [FK, 128, 128], BF16, kind="ExternalInput")
    s_in = nc.dram_tensor("s_in", [5, 128, WW], BF16, kind="ExternalInput")
    ct_in = nc.dram_tensor("ct_in", [128, WW], BF16, kind="ExternalInput")
    shw_in = nc.dram_tensor("shw_in", [6, 128, WW], BF16,
                            kind="ExternalInput")
    sc_in = nc.dram_tensor("sc_in", [1, 128], F32, kind="ExternalInput")
    on1_in = nc.dram_tensor("on1_in", [1, 128], F32, kind="ExternalInput")
    on128_in = nc.dram_tensor("on128_in", [128, 128], F32,
                              kind="ExternalInput")
    x_out = nc.dram_tensor("x_out", [C, IM, IM], F32, kind="ExternalOutput")

    RG = [[0, 1, 2, 3], [4, 5, 6, 7]]

    def mark(label):
        _PHASES.append((label, int(nc.get_next_instruction_name()[2:])))

    with tile.TileContext(nc) as tc:
        with (
            tc.tile_pool(name="const", bufs=1) as cpool,
            tc.tile_pool(name="state", bufs=1) as spool,
            tc.tile_pool(name="tmp", bufs=2) as tpool,
            tc.tile_pool(name="stg", bufs=2) as stgpool,
            tc.tile_pool(name="pwin", bufs=2, space="PSUM") as pwin,
            tc.tile_pool(name="pwb", bufs=2, space="PSUM") as pwb,
            tc.tile_pool(name="pda", bufs=2, space="PSUM") as pda,
            tc.tile_pool(name="pby", bufs=1, space="PSUM") as pby,
            tc.tile_pool(name="pbx", bufs=1, space="PSUM") as pbx,
            tc.tile_pool(name="dram", bufs=1, space="DRAM") as dpool,
        ):
            # ---------------- constants
            W1, W2n = {}, {}
            for c in range(C):
                for g in range(2):
                    for dx in range(K7):
                        t1_ = cpool.tile([128, WW], BF16, tag=f"w1_{c}_{g}_{dx}")
                        nc.sync.dma_start(t1_[:], w1_in[c, g, dx])
                        W1[c, g, dx] = t1_
                        t2_ = cpool.tile([WW, 128], BF16, tag=f"w2_{c}_{g}_{dx}")
                        nc.sync.dma_start(t2_[:], w2_in[c, g, dx])
                        W2n[c, g, dx] = t2_
            Wy = cpool.tile([128, 128], BF16, tag="wy")
            nc.sync.dma_start(Wy[:], wy_in[:])
            Cu = cpool.tile([128, 128], BF16, tag="cu")
            nc.sync.dma_start(Cu[:], cu_in[:])
            Cd = cpool.tile([128, 128], BF16, tag="cd")
            nc.sync.dma_start(Cd[:], cd_in[:])
            Dg = {}
            for dx in range(FK):
                t_ = cpool.tile([128, 128], BF16, tag=f"dg_{dx}")
                nc.sync.dma_start(t_[:], dg_in[dx])
                Dg[dx] = t_
            Sm = {}
            for j in range(5):
                t_ = cpool.tile([128, WW], BF16, tag=f"sm_{j}")
                nc.sync.dma_start(t_[:], s_in[j])
                Sm[j] = t_
            SVAR = {0: 0, 4: 1, 1: 2, 5: 2, 2: 3, 6: 3, 3: 4, 7: 4}
            Ct = cpool.tile([128, WW], BF16, tag="ct")
            nc.sync.dma_start(Ct[:], ct_in[:])
            SH = {}
            for v in range(6):
                t_ = cpool.tile([128, WW], BF16, tag=f"shw_{v}")
                nc.sync.dma_start(t_[:], shw_in[v])
                SH[v] = t_
            ones1 = cpool.tile([1, 128], F32, tag="ones1")
            nc.sync.dma_start(ones1[:], on1_in[:])
            ones128 = cpool.tile([128, 128], F32, tag="ones128")
            nc.sync.dma_start(ones128[:], on128_in[:])
            sc_raw = cpool.tile([1, 128], F32, tag="sc_raw")
            nc.sync.dma_start(sc_raw[:], sc_in[:])
            sc_ps = pbx.tile([128, 512], F32, tag="bx")
            nc.tensor.matmul(sc_ps[:, 0:128], ones1[:], sc_raw[:],
                             start=True, stop=True)
            sc_b = cpool.tile([128, 128], F32, tag="sc_b")
            nc.vector.tensor_copy(sc_b[:], sc_ps[:, 0:128])

            def scap(k, j):
                return sc_b[:, k * NSC + j:k * NSC + j + 1]

            # ---------------- state
            # a: bf16 margined [128=(4p,32y), 8 tiles x 262]
            A = {}
            for c in range(C):
                for g in range(2):
                    ta = spool.tile([128, NT * X], BF16, tag=f"a_{c}_{g}")
                    nc.vector.memset(ta[:], 0.0)
                    for p in range(4):
                        dst = ta[32 * p:32 * p + 32, :].rearrange(
                            "y (t x) -> y t x", x=X)[:, :, 3:259]
                        src = a_in[4 * g + p, c].rearrange(
                            "(t y) x -> y t x", y=32)
                        nc.sync.dma_start(dst, src)
                    A[c, g] = ta

            xA = spool.tile([128, 1536], F32, tag="xA")
            xB = spool.tile([128, 1536], F32, tag="xB")
            xbf = spool.tile([128, 1536], BF16, tag="xbf")
            y1a = spool.tile([128, 1536], F32, tag="y1a")
            y1b = spool.tile([128, 1536], F32, tag="y1b")
            nc.vector.memset(y1a[:], 0.0)
            nc.vector.memset(y1b[:], 0.0)
            y2m = spool.tile([128, 1536], BF16, tag="y2m")
            nc.vector.memset(y2m[:], 0.0)
            vm = spool.tile([128, 1536], BF16, tag="vm")
            zbf = spool.tile([128, 1536], BF16, tag="zbf")
            zt = spool.tile([128, 1536], F32, tag="zt")
            rn = spool.tile([128, 1536], BF16, tag="rn")
            rwin = spool.tile([WW, C * NT * X], BF16, tag="rwin")
            nc.vector.memset(rwin[:], 0.0)
            d0t = spool.tile([128, 1536], F32, tag="d0t")
            dvt = spool.tile([128, 1536], BF16, tag="dvt")
            dty = spool.tile([128, 1536], F32, tag="dty")
            tt1 = spool.tile([128, 1536], F32, tag="tt1")
            tt2 = spool.tile([128, 1536], F32, tag="tt2")

            src = zr_in[:].rearrange("c (h p) x -> p c h x", h=2)
            dst = zbf[:].rearrange("p (c h x) -> p c h x", c=C, h=2)
            nc.sync.dma_start(dst, src)
            src = zf_in[:].rearrange("c (h p) x -> p c h x", h=2)
            dst = zt[:].rearrange("p (c h x) -> p c h x", c=C, h=2)
            nc.sync.dma_start(dst, src)

            bnci, bnco = {}, {}
            for c in range(C):
                bnci[c] = dpool.tile([IM, IM], BF16, tag=f"bnci_{c}",
                                     name=f"bnci_{c}")
                bnco[c] = dpool.tile([IM, IM], BF16, tag=f"bnco_{c}",
                                     name=f"bnco_{c}")

            def cblk(t, c):
                return t[:, 512 * c:512 * c + 512]

            def chx(t, c, h):
                return t[:, 512 * c + 256 * h:512 * c + 256 * h + 256]

            def pair2(t, c):
                # [128, 2, 256] over the two h blocks of channel c
                return t[:, 512 * c:512 * c + 512].rearrange(
                    "p (h x) -> p h x", x=256)

            # blur of one channel: src bf16 [128,1536] -> consume(pxm)
            def blur_c(src, c, consume):
                py = pby.tile([128, 512], F32, tag="by")
                nc.tensor.matmul(py[:], Wy[:], pair2(src, c),
                                 start=True, stop=False)
                nc.tensor.matmul(py[:, 0:256], Cu[:],
                                 src[:, 512 * c + 256:512 * c + 512],
                                 start=False, stop=False)
                nc.tensor.matmul(py[:, 256:512], Cd[:],
                                 src[:, 512 * c:512 * c + 256],
                                 start=False, stop=True)
                pyb = tpool.tile([128, 520], BF16, tag="pyb")
                nc.vector.memset(pyb[:, 0:2], 0.0)
                nc.vector.memset(pyb[:, 258:262], 0.0)
                nc.vector.memset(pyb[:, 518:520], 0.0)
                pv = pyb[:].rearrange("p (h x) -> p h x", x=260)
                nc.scalar.copy(pv[:, :, 2:258],
                               py[:].rearrange("p (h x) -> p h x", x=256))
                pxm = pbx.tile([128, 512], F32, tag="bx")
                for dx in range(FK):
                    nc.tensor.matmul(
                        pxm[:], Dg[dx], pv[:, :, dx:dx + 256],
                        start=(dx == 0), stop=(dx == FK - 1))
                consume(pxm)

            # x0 = blur(z)
            for c in range(C):
                def init_consume(pxm, c=c):
                    nc.vector.tensor_copy(cblk(xA, c), pxm[:])
                    nc.scalar.copy(cblk(xbf, c), pxm[:])

                blur_c(zbf, c, init_consume)

            xcur, xnxt = xA, xB
            for k in range(KS):
                # ---- conv_CSC + (-x/4) + stage + AR, per channel
                for c in range(C):
                    mark(f'k{k}_conv{c}')
                    st = stgpool.tile([WW, NT * 256], BF16, tag="stage")
                    for pi in range(4):
                        pq = pwin.tile([WW, 512], F32, tag="baq")
                        first = True
                        for g in range(2):
                            av = A[c, g][:].rearrange(
                                "p (t x) -> p t x", x=X)
                            for dx in range(K7):
                                mv = av[:, 2 * pi:2 * pi + 2,
                                        dx:dx + 256]
                                nc.tensor.matmul(pq[:], W1[c, g, dx], mv,
                                                 start=first, stop=False)
                                first = False
                        for ti in range(2):
                            t = 2 * pi + ti
                            h = t // 4
                            col = 256 * ti
                            last = (ti == 1) and t != 3
                            nc.tensor.matmul(
                                pq[0:WW, col:col + 256], Sm[SVAR[t]],
                                xbf[:, 512 * c + 256 * h:
                                    512 * c + 256 * h + 256],
                                start=False, stop=last)
                            if t == 3:
                                nc.tensor.matmul(
                                    pq[0:WW, col:col + 256], Ct,
                                    xbf[:, 512 * c + 256:512 * c + 512],
                                    start=False, stop=True)
                        nc.scalar.copy(st[:, 512 * pi:512 * pi + 512],
                                       pq[:])
                        # overlap-add: rows 32:38 of window t gain rows
                        # 0:6 of window t+1 (same psum pair for even t,
                        # previous pair's carry for odd t)
                        t0, t1_ = 2 * pi, 2 * pi + 1
                        nc.vector.tensor_add(
                            st[32:38, 256 * t0:256 * t0 + 256],
                            st[32:38, 256 * t0:256 * t0 + 256],
                            pq[0:6, 256:512])
                        if pi >= 1:
                            tp = 2 * pi - 1
                            nc.vector.tensor_add(
                                st[32:38, 256 * tp:256 * tp + 256],
                                st[32:38, 256 * tp:256 * tp + 256],
                                pq[0:6, 0:256])
                    # 3 DMAs -> bnci[c]
                    dstA = bnci[c][3:227, :].rearrange(
                        "(q r) x -> r q x", r=32)
                    srcA = st[6:38, 0:7 * 256].rearrange(
                        "r (q x) -> r q x", x=256)
                    nc.sync.dma_start(dstA, srcA)
                    nc.sync.dma_start(bnci[c][227:256, :],
                                      st[6:35, 7 * 256:8 * 256])
                    nc.sync.dma_start(bnci[c][0:3, :], st[3:6, 0:256])
                    nc.gpsimd.collective_compute(
                        "AllReduce", AX.add, replica_groups=RG,
                        ins=[bnci[c].opt()], outs=[bnco[c].opt()])

                # ---- load -r as soon as each AR lands
                mark(f'k{k}_loads')
                for c in range(C):
                    nc.sync.dma_start(
                        cblk(rn, c).rearrange("p (h x) -> p h x", x=256),
                        bnco[c][:].rearrange("(h p) x -> p h x", h=2))

                # ---- window build (PE) + convT + a update
                for c in range(C):
                    mark(f'k{k}_convT{c}')
                    # extract the 8 windows of -r for this channel from rn
                    # via shift matmuls; scalar-copy psum -> rwin interior
                    rwv = rwin[:].rearrange("r (t x) -> r t x", x=X)
                    for pi in range(4):
                        pw = pwb.tile([WW, 512], F32, tag="wb")
                        for ti in range(2):
                            t = 2 * pi + ti
                            j, h = t % 4, t // 4
                            col = 256 * ti
                            nc.tensor.matmul(
                                pw[0:WW, col:col + 256], SH[j],
                                rn[:, 512 * c + 256 * h:
                                   512 * c + 256 * h + 256],
                                start=True, stop=(t not in (3, 4)))
                            if t == 3:
                                nc.tensor.matmul(
                                    pw[0:WW, col:col + 256], SH[4],
                                    rn[:, 512 * c + 256:512 * c + 512],
                                    start=False, stop=True)
                            if t == 4:
                                nc.tensor.matmul(
                                    pw[0:WW, col:col + 256], SH[5],
                                    rn[:, 512 * c:512 * c + 256],
                                    start=False, stop=True)
                        dstP = rwv[:, NT * c + 2 * pi:NT * c + 2 * pi + 2,
                                   3:259]
                        nc.scalar.copy(
                            dstP, pw[:].rearrange("r (b x) -> r b x", x=256))
                    for g in range(2):
                        for cc in range(2):
                            tav = tpool.tile([128, 1024], F32, tag="tav")
                            for pp in range(2):
                                p2 = 2 * cc + pp
                                pd = pda.tile([128, 512], F32, tag="da")
                                mv = rwv[:, NT * c + 2 * p2:
                                         NT * c + 2 * p2 + 2, :]
                                for i, dx in enumerate(range(K7)):
                                    nc.tensor.matmul(
                                        pd[:], W2n[c, g, dx],
                                        mv[:, :, dx:dx + 256],
                                        start=(i == 0), stop=(i == K7 - 1))
                                av = A[c, g][:].rearrange(
                                    "p (t x) -> p t x", x=X)[
                                    :, 2 * p2:2 * p2 + 2, 3:259]
                                nc.vector.scalar_tensor_tensor(
                                    tav[:, 512 * pp:512 * pp + 512].rearrange(
                                        "p (b x) -> p b x", x=256),
                                    pd[:].rearrange("p (b x) -> p b x", x=256),
                                    scap(k, 1), av,
                                    op0=AX.mult, op1=AX.add)
                            s1 = tpool.tile([128, 1024], F32, tag="s1")
                            s2 = tpool.tile([128, 1024], F32, tag="s2")
                            nc.scalar.activation(s1[:], tav[:], AF.Relu,
                                                 bias=scap(k, 2), scale=1.0)
                            nc.scalar.activation(s2[:], tav[:], AF.Relu,
                                                 bias=scap(k, 2), scale=-1.0)
                            av4 = A[c, g][:].rearrange(
                                "p (t x) -> p t x", x=X)[
                                :, 4 * cc:4 * cc + 4, 3:259]
                            nc.vector.tensor_sub(
                                av4,
                                s1[:].rearrange("p (t x) -> p t x", x=256),
                                s2[:].rearrange("p (t x) -> p t x", x=256))

                # ---- deferred prox of the previous step (overlaps the
                # conv/convT PE work of this step)
                if k >= 1:
                    mark(f'k{k}_prox')
                    kp = k - 1
                    nc.scalar.square(tt1[:], y1a[:])
                    nc.scalar.square(tt2[:], y1b[:])
                    nc.vector.tensor_add(tt1[:], tt1[:], tt2[:])
                    nc.scalar.activation(tt2[:], tt1[:], AF.Ln,
                                         bias=scap(kp, 3), scale=1.0)
                    nc.scalar.activation(tt1[:], tt2[:], AF.Exp,
                                         bias=0.0, scale=-0.5)
                    nc.scalar.activation(tt2[:], tt1[:], AF.Relu,
                                         bias=1.0, scale=scap(kp, 5))
                    nc.scalar.activation(tt1[:], tt2[:], AF.Identity,
                                         bias=1.0, scale=scap(kp, 6))
                    nc.vector.tensor_mul(y1a[:], y1a[:], tt1[:])
                    nc.vector.tensor_mul(y1b[:], y1b[:], tt1[:])
                    nc.vector.scalar_tensor_tensor(
                        tt1[:], y2m[:], scap(kp, 7), zt[:],
                        op0=AX.mult, op1=AX.subtract)
                    nc.scalar.square(tt2[:], tt1[:])
                    nrs = tpool.tile([128, 1], F32, tag="nrs")
                    nc.vector.tensor_reduce(nrs[:], tt2[:], axis=AXL.X,
                                            op=AX.add)
                    tot = pbx.tile([128, 512], F32, tag="bx")
                    nc.tensor.matmul(tot[:, 0:1], ones128[:], nrs[:],
                                     start=True, stop=True)
                    fsc = tpool.tile([128, 1], F32, tag="fsc")
                    nc.scalar.sqrt(fsc[:], tot[:, 0:1])
                    nc.vector.tensor_scalar_max(fsc[:], fsc[:], float(EPS))
                    rc2 = tpool.tile([128, 1], F32, tag="rc2")
                    nc.vector.reciprocal(rc2[:], fsc[:])
                    nc.vector.tensor_scalar(fsc[:], rc2[:], EPS_R, 1.0,
                                            op0=AX.mult, op1=AX.min)
                    nc.vector.scalar_tensor_tensor(
                        tt2[:], tt1[:], fsc[:], zt[:],
                        op0=AX.mult, op1=AX.add)
                    for c in range(C):
                        nc.vector.scalar_tensor_tensor(
                            cblk(y2m, c), cblk(tt2, c), scap(kp, 8),
                            cblk(y2m, c), op0=AX.mult, op1=AX.add)

                # ---- dty = Dt(y1) + blur(y2) - r, then x update + v
                mark(f'k{k}_dty')
                nc.sync.dma_start(d0t[1:128, :], y1a[0:127, :])
                for h in range(2):
                    dsh = d0t[0:1, :].rearrange(
                        "p (c h x) -> p c h x", c=C, h=2)[:, :, h:h + 1, :]
                    ssh = y1a[127:128, :].rearrange(
                        "p (c h x) -> p c h x", c=C, h=2)[:, :, 1 - h:2 - h, :]
                    nc.sync.dma_start(dsh, ssh)
                nc.vector.tensor_sub(dty[:], d0t[:], y1a[:])
                v6o = tt1[:].rearrange("p (b x) -> p b x", x=256)
                v6i = y1b[:].rearrange("p (b x) -> p b x", x=256)
                nc.vector.tensor_sub(v6o[:, :, 1:256], v6i[:, :, 0:255],
                                     v6i[:, :, 1:256])
                nc.vector.tensor_sub(v6o[:, :, 0:1], v6i[:, :, 255:256],
                                     v6i[:, :, 0:1])
                nc.vector.tensor_add(dty[:], dty[:], tt1[:])

                for c in range(C):
                    def y2blur_consume(pxm, c=c):
                        nc.vector.tensor_add(cblk(dty, c), cblk(dty, c),
                                             pxm[:])

                    blur_c(y2m, c, y2blur_consume)
                    nc.vector.tensor_sub(cblk(dty, c), cblk(dty, c),
                                         cblk(rn, c))
                    nc.vector.scalar_tensor_tensor(
                        cblk(tt2, c), cblk(dty, c), scap(k, 0), cblk(xcur, c),
                        op0=AX.mult, op1=AX.add)
                    nc.vector.tensor_scalar(cblk(xnxt, c), cblk(tt2, c),
                                            0.0, 1.0, op0=AX.max, op1=AX.min)
                    nc.vector.scalar_tensor_tensor(
                        cblk(vm, c), cblk(xnxt, c), 2.0, cblk(xcur, c),
                        op0=AX.mult, op1=AX.subtract)
                    nc.scalar.copy(cblk(xbf, c), cblk(xnxt, c))

                    def y2_consume(pxm, c=c, k=k):
                        nc.vector.scalar_tensor_tensor(
                            cblk(y2m, c), pxm[:], scap(k, 4), cblk(y2m, c),
                            op0=AX.mult, op1=AX.add)

                    blur_c(vm, c, y2_consume)

                # ---- y1/y2 accumulate (prox deferred to next iteration)
                mark(f'k{k}_acc')
                nc.sync.dma_start(dvt[0:127, :], vm[1:128, :])
                for h in range(2):
                    dsh = dvt[127:128, :].rearrange(
                        "p (c h x) -> p c h x", c=C, h=2)[:, :, h:h + 1, :]
                    ssh = vm[0:1, :].rearrange(
                        "p (c h x) -> p c h x", c=C, h=2)[:, :, 1 - h:2 - h, :]
                    nc.sync.dma_start(dsh, ssh)
                nc.vector.tensor_sub(tt1[:], dvt[:], vm[:])
                nc.vector.scalar_tensor_tensor(
                    y1a[:], tt1[:], scap(k, 4), y1a[:],
                    op0=AX.mult, op1=AX.add)
                v6o = tt2[:].rearrange("p (b x) -> p b x", x=256)
                v6i = vm[:].rearrange("p (b x) -> p b x", x=256)
                nc.vector.tensor_sub(v6o[:, :, 0:255], v6i[:, :, 1:256],
                                     v6i[:, :, 0:255])
                nc.vector.tensor_sub(v6o[:, :, 255:256], v6i[:, :, 0:1],
                                     v6i[:, :, 255:256])
                nc.vector.scalar_tensor_tensor(
                    y1b[:], tt2[:], scap(k, 4), y1b[:],
                    op0=AX.mult, op1=AX.add)

                xcur, xnxt = xnxt, xcur

            # ---------------- output
            dstO = x_out[:].rearrange("c (h p) x -> p c h x", h=2)
            srcO = xcur[:].rearrange("p (c h x) -> p c h x", c=C, h=2)
            nc.sync.dma_start(dstO, srcO)

    _split_sync_waits(nc)
    return nc


# ---------------------------------------------------------------- host entry


def kernel(z, a_init, B, fil, lam1, lam2, gam1, gam2, gam3):
    from concourse.bass_utils import run_bass_kernel_spmd

    if "nc" not in _NC_CACHE:
        _NC_CACHE["nc"] = _build_nc()
    nc = _NC_CACHE["nc"]

    z = np.asarray(z, np.float32)
    a_init = np.asarray(a_init, np.float32)
    B = np.asarray(B, np.float32)
    fil = np.asarray(fil, np.float32)
    sc = _build_sc(np.asarray(lam1, np.float32), np.asarray(lam2, np.float32),
                   np.asarray(gam1, np.float32), np.asarray(gam2, np.float32),
                   np.asarray(gam3, np.float32))
    Wy, Cu, Cd, Dg = _build_sep_blur(fil)
    Sm, Ct = _build_shift_s()
    Shw = _build_shift_w()
    on1 = np.ones((1, 128), np.float32)
    on128 = np.ones((128, 128), np.float32)

    in_maps = []
    for core in range(8):
        b, pg = core // 4, core % 4
        Bsh = B[PL * pg:PL * pg + PL]
        in_maps.append({
            "a_in": np.ascontiguousarray(
                a_init[b, PL * pg:PL * pg + PL]).astype(BF),
            "zr_in": np.ascontiguousarray(z[b]).astype(BF),
            "zf_in": np.ascontiguousarray(z[b]),
            "w1_in": _build_w1(Bsh),
            "w2_in": _build_w2n(Bsh),
            "wy_in": Wy, "cu_in": Cu, "cd_in": Cd, "dg_in": Dg,
            "s_in": Sm, "ct_in": Ct, "shw_in": Shw,
            "sc_in": sc, "on1_in": on1, "on128_in": on128,
        })
    global _LAST_IN_MAPS
    _LAST_IN_MAPS = in_maps
    res = run_bass_kernel_spmd(nc, in_maps, core_ids=list(range(8)))
    out = np.stack([res.results[0]["x_out"], res.results[4]["x_out"]])
    return out.astype(np.float32)


# revision 54
# speedup vs baseline: 1.1772x; 1.0651x over previous
"""CSC-TV primal-dual solver on 8 Trainium2 NeuronCores (v2, bf16 PE path).

Sharding: mb(2) x p-groups(4 of 8 filters) = 8 cores. Each core holds one
batch sample's full x/y1/y2 state (replicated within its mb-group of 4
cores) plus an 8-filter shard of `a`.

v2 design:
- All matmuls bf16 (states that feed the PE kept as bf16; x/y1 and DVE
  accumulators stay f32). N=512 matmuls via 3D access patterns.
- The AllReduce payload is (Ba_part - x/4) in bf16, so the reduced
  result is -(x - Ba) = -r directly: censored shift matmuls add -x/4
  into the conv PSUM windows. convT windows then load straight from the
  bounce buffer with 3 DMAs/channel; W2 is sign-flipped to absorb -r.
- One AllReduce per channel, pipelined against conv/convT of the other
  channels.
- Separable (rank-1) blur: banded y-pass matmul + corner matmuls from
  32-aligned windows, then 5 diagonal x-shift matmuls.
- Soft-threshold via two scalar-engine Relu passes; sqrt on scalar;
  reciprocal via the fast DVE approximation.
"""
import numpy as np
import ml_dtypes

EPS = 1e-8
ALPHA = 0.05
KS = 10          # iterations
C = 3
IM = 256         # image side
K7 = 7
FK = 5
PL = 8           # filters per core
X = 262          # 256 + 2*3 x-margins
NT = 8           # 32-row y tiles
WW = 38          # conv out-window rows
NSC = 10         # scalar slots per step
EPS_R = float(ALPHA * np.sqrt(np.float32(C * IM * IM)))

_NC_CACHE = {}
_LAST_IN_MAPS = None
_PHASES = []

# ---------------------------------------------------------------- walrus fixes


def _apply_walrus_workarounds():
    import concourse.tile as tile
    from concourse.vector_clock import ScopedClock, VectorClock

    def _chunked_drain_and_barrier(self, tick_clock, wait_clock):
        vec = list(tick_clock.global_clock)
        for i, tick in enumerate(vec):
            if tick <= 0:
                continue
            sub = [0] * len(vec)
            sub[i] = tick
            drain_inst = self.nc.sync.drain()
            wait_clock.add_sem_waits(
                drain_inst.ins, ScopedClock({None: VectorClock(sub)}))
        self.nc.all_engine_barrier()
        assert self.sems is not None
        popped = self.nc._tile_sem_poison_stack.pop()
        assert popped is self._sem_poison
        self.nc.clear_and_free_semaphores(
            list(self.sems.allocated().values()))
        self.nc.all_engine_barrier()

    tile.TileContext._drain_and_barrier = _chunked_drain_and_barrier


def _split_sync_waits(nc):
    """This walrus build allows a single sync-wait command per
    instruction; hoist extras onto same-engine no-ops."""
    from concourse import mybir
    for fn in nc.m.functions:
        for bb in fn.blocks:
            out = []
            for ins in bb.instructions:
                si = ins.sync_info
                if si is not None and si.on_wait and len(si.on_wait) > 1:
                    waits = list(si.on_wait)
                    extra, keep = waits[:-1], waits[-1:]
                    for k, w in enumerate(extra):
                        out.append(mybir.InstNoOp(
                            name=f"{ins.name}-ws{k}",
                            sync_info=mybir.SyncInfo(
                                on_wait=[w], on_update=[]),
                            bass_nofuse=True,
                            engine=ins.engine))
                    ins.sync_info = mybir.SyncInfo(
                        on_wait=keep, on_update=list(si.on_update))
                out.append(ins)
            try:
                bb.instructions = out
            except Exception:
                bb.instructions.clear()
                for i in out:
                    bb.instructions.append(i)


# ---------------------------------------------------------------- band builders

BF = ml_dtypes.bfloat16


def _build_w1(Bsh):
    # Bsh: (PL, C, 7, 7). W1[c,g,dx]: [128=(p4,y32), 38]
    W1 = np.zeros((C, 2, K7, 128, WW), np.float32)
    yi = np.arange(32)[:, None]
    m = np.arange(WW)[None, :]
    dy = yi - m + 6                       # [32, WW]
    msk = (dy >= 0) & (dy < 7)
    dyc = np.clip(dy, 0, 6)
    for c in range(C):
        for g in range(2):
            for dx in range(K7):
                for p in range(4):
                    vals = Bsh[4 * g + p, c, dyc, dx] * msk
                    W1[c, g, dx, 32 * p:32 * p + 32, :] = vals
    return W1.astype(BF)


def _build_w2n(Bsh):
    # W2n[c,g,dx]: [38, 128], sign-flipped adjoint band
    W2 = np.zeros((C, 2, K7, WW, 128), np.float32)
    yi = np.arange(32)[None, :]
    k = np.arange(WW)[:, None]
    dy = k - yi                           # [WW, 32]
    msk = (dy >= 0) & (dy < 7)
    dyc = np.clip(dy, 0, 6)
    for c in range(C):
        for g in range(2):
            for dx in range(K7):
                for p in range(4):
                    vals = Bsh[4 * g + p, c, 6 - dyc, 6 - dx] * msk
                    W2[c, g, dx, :, 32 * p:32 * p + 32] = vals
    return W2.astype(BF)


def _build_sep_blur(fil):
    # fil rank-1: fil = outer(gc, gr)
    u, s, vt = np.linalg.svd(fil.astype(np.float64))
    gc = (u[:, 0] * np.sqrt(s[0]))
    gr = (vt[0] * np.sqrt(s[0]))
    if gc[FK // 2] < 0:
        gc, gr = -gc, -gr
    gc = gc.astype(np.float32)
    gr = gr.astype(np.float32)
    # y-pass band: out[m] = sum_d gc[d] * in[m+d-2]
    Wy = np.zeros((128, 128), np.float32)
    kk = np.arange(128)[:, None]
    m = np.arange(128)[None, :]
    d = kk - m + 2
    msk = (d >= 0) & (d < FK)
    Wy[msk] = gc[np.clip(d, 0, FK - 1)][msk]
    # corner up: moving = full next block; rows 0,1 feed this block out 126,127
    Cu = np.zeros((128, 128), np.float32)
    for p in range(128):
        for mm in range(128):
            dd = 128 + p - mm + 2
            if 0 <= dd < FK:
                Cu[p, mm] = gc[dd]
    # corner down: moving = full prev block; rows 126,127 feed next out 0,1
    Cd = np.zeros((128, 128), np.float32)
    for p in range(128):
        for mm in range(128):
            dd = p - (128 + mm) + 2
            if 0 <= dd < FK:
                Cd[p, mm] = gc[dd]
    # x-pass diagonals
    Dg = np.zeros((FK, 128, 128), np.float32)
    for dx in range(FK):
        Dg[dx] = np.eye(128, dtype=np.float32) * gr[dx]
    return Wy.astype(BF), Cu.astype(BF), Cd.astype(BF), Dg.astype(BF)


def _build_shift_s():
    # Window t places -0.25*x at window-row m (= image row 32t-3+m) only
    # for the rows it "owns": m>=6 (m>=3 for t=0), so stitched overlaps
    # get the x-term exactly once. p = 32j-3+m into the moving x block.
    # Variants: 0: t=0 (j=0, m>=3); 1: t=4 (j=0, m>=6); 2: j=1; 3: j=2;
    # 4: j=3 (m>=6; m>=35 spills to the Ct corner, emitted for t=3 only).
    S = np.zeros((5, 128, WW), np.float32)
    specs = [(0, 3), (0, 6), (1, 6), (2, 6), (3, 6)]
    for v, (j, mlo) in enumerate(specs):
        for m in range(mlo, WW):
            p = 32 * j - 3 + m
            if 0 <= p < 128:
                S[v, p, m] = -0.25
    # tail corner (window t=3): image rows 128..130 = block h1 parts 0..2
    Ct = np.zeros((128, WW), np.float32)
    for p in range(3):
        Ct[p, 35 + p] = -0.25
    return S.astype(BF), Ct.astype(BF)


def _build_shift_w():
    # Window-extraction matrices: window t rows m <- block partition
    # p = 32(t%4)-3+m. Variants 0..3 = t%4; 4 = t=3 tail (h1 rows 0..2);
    # 5 = t=4 head (h0 rows 125..127).
    Sh = np.zeros((6, 128, WW), np.float32)
    for j in range(4):
        for m in range(WW):
            p = 32 * j - 3 + m
            if 0 <= p < 128:
                Sh[j, p, m] = 1.0
    for p in range(3):
        Sh[4, p, 35 + p] = 1.0
    for p in range(125, 128):
        Sh[5, p, p - 125] = 1.0
    return Sh.astype(BF)


def _build_sc(lam1, lam2, gam1, gam2, gam3):
    sc = np.zeros((1, 128), np.float32)
    for k in range(KS):
        g1 = np.float32(gam1[k])
        g2 = np.float32(gam2[k])
        g3 = np.float32(gam3[k])
        l1, l2 = np.float32(lam1[k]), np.float32(lam2[k])
        g3e = np.float32(g3 + np.float32(EPS))
        o = k * NSC
        sc[0, o + 0] = -g1
        sc[0, o + 1] = g2
        sc[0, o + 2] = -(g2 * l1)
        sc[0, o + 3] = (np.float32(EPS) * g3e) ** 2
        sc[0, o + 4] = g3
        sc[0, o + 5] = -l2
        sc[0, o + 6] = -(g3 / g3e)
        sc[0, o + 7] = np.float32(1.0) / g3e
        sc[0, o + 8] = -g3
        sc[0, o + 9] = np.float32(EPS) * g3e
    return sc


# ---------------------------------------------------------------- device build


def _build_nc():
    import concourse.bass as bass
    import concourse.mybir as mybir
    import concourse.tile as tile

    _apply_walrus_workarounds()

    F32 = mybir.dt.float32
    BF16 = mybir.dt.bfloat16
    AX = mybir.AluOpType
    AF = mybir.ActivationFunctionType
    AXL = mybir.AxisListType

    nc = bass.Bass()
    a_in = nc.dram_tensor("a_in", [PL, C, IM, IM], BF16, kind="ExternalInput")
    zr_in = nc.dram_tensor("zr_in", [C, IM, IM], BF16, kind="ExternalInput")
    zf_in = nc.dram_tensor("zf_in", [C, IM, IM], F32, kind="ExternalInput")
    w1_in = nc.dram_tensor("w1_in", [C, 2, K7, 128, WW], BF16,
                           kind="ExternalInput")
    w2_in = nc.dram_tensor("w2_in", [C, 2, K7, WW, 128], BF16,
                           kind="ExternalInput")
    wy_in = nc.dram_tensor("wy_in", [128, 128], BF16, kind="ExternalInput")
    cu_in = nc.dram_tensor("cu_in", [128, 128], BF16, kind="ExternalInput")
    cd_in = nc.dram_tensor("cd_in", [128, 128], BF16, kind="ExternalInput")
    dg_in = nc.dram_tensor("dg_in", [FK, 128, 128], BF16, kind="ExternalInput")
    shw_in = nc.dram_tensor("shw_in", [6, 128, WW], BF16,
                            kind="ExternalInput")
    sc_in = nc.dram_tensor("sc_in", [1, 128], F32, kind="ExternalInput")
    on1_in = nc.dram_tensor("on1_in", [1, 128], F32, kind="ExternalInput")
    on128_in = nc.dram_tensor("on128_in", [128, 128], F32,
                              kind="ExternalInput")
    x_out = nc.dram_tensor("x_out", [C, IM, IM], F32, kind="ExternalOutput")

    RG = [[0, 1, 2, 3], [4, 5, 6, 7]]

    def mark(label):
        _PHASES.append((label, int(nc.get_next_instruction_name()[2:])))

    with tile.TileContext(nc) as tc:
        with (
            tc.tile_pool(name="const", bufs=1) as cpool,
            tc.tile_pool(name="state", bufs=1) as spool,
            tc.tile_pool(name="tmp", bufs=2) as tpool,
            tc.tile_pool(name="stg", bufs=2) as stgpool,
            tc.tile_pool(name="pwin", bufs=2, space="PSUM") as pwin,
            tc.tile_pool(name="pwb", bufs=2, space="PSUM") as pwb,
            tc.tile_pool(name="pda", bufs=2, space="PSUM") as pda,
            tc.tile_pool(name="pby", bufs=1, space="PSUM") as pby,
            tc.tile_pool(name="pbx", bufs=1, space="PSUM") as pbx,
            tc.tile_pool(name="dram", bufs=1, space="DRAM") as dpool,
        ):
            # ---------------- constants
            W1, W2n = {}, {}
            for c in range(C):
                for g in range(2):
                    for dx in range(K7):
                        t1_ = cpool.tile([128, WW], BF16, tag=f"w1_{c}_{g}_{dx}")
                        nc.sync.dma_start(t1_[:], w1_in[c, g, dx])
                        W1[c, g, dx] = t1_
                        t2_ = cpool.tile([WW, 128], BF16, tag=f"w2_{c}_{g}_{dx}")
                        nc.sync.dma_start(t2_[:], w2_in[c, g, dx])
                        W2n[c, g, dx] = t2_
            Wy = cpool.tile([128, 128], BF16, tag="wy")
            nc.sync.dma_start(Wy[:], wy_in[:])
            Cu = cpool.tile([128, 128], BF16, tag="cu")
            nc.sync.dma_start(Cu[:], cu_in[:])
            Cd = cpool.tile([128, 128], BF16, tag="cd")
            nc.sync.dma_start(Cd[:], cd_in[:])
            Dg = {}
            for dx in range(FK):
                t_ = cpool.tile([128, 128], BF16, tag=f"dg_{dx}")
                nc.sync.dma_start(t_[:], dg_in[dx])
                Dg[dx] = t_
            SH = {}
            for v in range(6):
                t_ = cpool.tile([128, WW], BF16, tag=f"shw_{v}")
                nc.sync.dma_start(t_[:], shw_in[v])
                SH[v] = t_
            ones1 = cpool.tile([1, 128], F32, tag="ones1")
            nc.sync.dma_start(ones1[:], on1_in[:])
            ones128 = cpool.tile([128, 128], F32, tag="ones128")
            nc.sync.dma_start(ones128[:], on128_in[:])
            sc_raw = cpool.tile([1, 128], F32, tag="sc_raw")
            nc.sync.dma_start(sc_raw[:], sc_in[:])
            sc_ps = pbx.tile([128, 512], F32, tag="bx")
            nc.tensor.matmul(sc_ps[:, 0:128], ones1[:], sc_raw[:],
                             start=True, stop=True)
            sc_b = cpool.tile([128, 128], F32, tag="sc_b")
            nc.vector.tensor_copy(sc_b[:], sc_ps[:, 0:128])

            def scap(k, j):
                return sc_b[:, k * NSC + j:k * NSC + j + 1]

            # ---------------- state
            # a: bf16 margined [128=(4p,32y), 8 tiles x 262]
            A = {}
            for c in range(C):
                for g in range(2):
                    ta = spool.tile([128, NT * X], BF16, tag=f"a_{c}_{g}")
                    nc.vector.memset(ta[:], 0.0)
                    for p in range(4):
                        dst = ta[32 * p:32 * p + 32, :].rearrange(
                            "y (t x) -> y t x", x=X)[:, :, 3:259]
                        src = a_in[4 * g + p, c].rearrange(
                            "(t y) x -> y t x", y=32)
                        nc.sync.dma_start(dst, src)
                    A[c, g] = ta

            xA = spool.tile([128, 1536], F32, tag="xA")
            xB = spool.tile([128, 1536], F32, tag="xB")
            rbf = spool.tile([128, 1536], BF16, tag="rbf")
            y1a = spool.tile([128, 1536], F32, tag="y1a")
            y1b = spool.tile([128, 1536], F32, tag="y1b")
            nc.vector.memset(y1a[:], 0.0)
            nc.vector.memset(y1b[:], 0.0)
            y2m = spool.tile([128, 1536], BF16, tag="y2m")
            nc.vector.memset(y2m[:], 0.0)
            vm = spool.tile([128, 1536], BF16, tag="vm")
            zbf = spool.tile([128, 1536], BF16, tag="zbf")
            zt = spool.tile([128, 1536], F32, tag="zt")
            rn = spool.tile([128, 1536], BF16, tag="rn")
            rwin = spool.tile([WW, C * NT * X], BF16, tag="rwin")
            nc.vector.memset(rwin[:], 0.0)
            d0t = spool.tile([128, 1536], F32, tag="d0t")
            dvt = spool.tile([128, 1536], BF16, tag="dvt")
            dty = spool.tile([128, 1536], F32, tag="dty")
            tt1 = spool.tile([128, 1536], F32, tag="tt1")
            tt2 = spool.tile([128, 1536], F32, tag="tt2")

            src = zr_in[:].rearrange("c (h p) x -> p c h x", h=2)
            dst = zbf[:].rearrange("p (c h x) -> p c h x", c=C, h=2)
            nc.sync.dma_start(dst, src)
            src = zf_in[:].rearrange("c (h p) x -> p c h x", h=2)
            dst = zt[:].rearrange("p (c h x) -> p c h x", c=C, h=2)
            nc.sync.dma_start(dst, src)

            bnci, bnco = {}, {}
            for c in range(C):
                bnci[c] = dpool.tile([IM, IM], BF16, tag=f"bnci_{c}",
                                     name=f"bnci_{c}")
                bnco[c] = dpool.tile([IM, IM], BF16, tag=f"bnco_{c}",
                                     name=f"bnco_{c}")

            def cblk(t, c):
                return t[:, 512 * c:512 * c + 512]

            def chx(t, c, h):
                return t[:, 512 * c + 256 * h:512 * c + 256 * h + 256]

            def pair2(t, c):
                # [128, 2, 256] over the two h blocks of channel c
                return t[:, 512 * c:512 * c + 512].rearrange(
                    "p (h x) -> p h x", x=256)

            # blur of one channel: src bf16 [128,1536] -> consume(pxm)
            def blur_c(src, c, consume):
                py = pby.tile([128, 512], F32, tag="by")
                nc.tensor.matmul(py[:], Wy[:], pair2(src, c),
                                 start=True, stop=False)
                nc.tensor.matmul(py[:, 0:256], Cu[:],
                                 src[:, 512 * c + 256:512 * c + 512],
                                 start=False, stop=False)
                nc.tensor.matmul(py[:, 256:512], Cd[:],
                                 src[:, 512 * c:512 * c + 256],
                                 start=False, stop=True)
                pyb = tpool.tile([128, 520], BF16, tag="pyb")
                nc.vector.memset(pyb[:, 0:2], 0.0)
                nc.vector.memset(pyb[:, 258:262], 0.0)
                nc.vector.memset(pyb[:, 518:520], 0.0)
                pv = pyb[:].rearrange("p (h x) -> p h x", x=260)
                nc.scalar.copy(pv[:, :, 2:258],
                               py[:].rearrange("p (h x) -> p h x", x=256))
                pxm = pbx.tile([128, 512], F32, tag="bx")
                for dx in range(FK):
                    nc.tensor.matmul(
                        pxm[:], Dg[dx], pv[:, :, dx:dx + 256],
                        start=(dx == 0), stop=(dx == FK - 1))
                consume(pxm)

            # x0 = blur(z)
            for c in range(C):
                def init_consume(pxm, c=c):
                    nc.vector.tensor_copy(cblk(xA, c), pxm[:])

                blur_c(zbf, c, init_consume)

            xcur, xnxt = xA, xB
            for k in range(KS):
                # ---- conv_CSC + (-x/4) + stage + AR, per channel
                for c in range(C):
                    mark(f'k{k}_conv{c}')
                    st = stgpool.tile([WW, NT * 256], BF16, tag="stage")
                    for pi in range(4):
                        pq = pwin.tile([WW, 512], F32, tag="baq")
                        first = True
                        for g in range(2):
                            av = A[c, g][:].rearrange(
                                "p (t x) -> p t x", x=X)
                            for dx in range(K7):
                                mv = av[:, 2 * pi:2 * pi + 2,
                                        dx:dx + 256]
                                nc.tensor.matmul(
                                    pq[:], W1[c, g, dx], mv,
                                    start=first,
                                    stop=(g == 1 and dx == K7 - 1))
                                first = False
                        nc.scalar.copy(st[:, 512 * pi:512 * pi + 512],
                                       pq[:])
                        # overlap-add: rows 32:38 of window t gain rows
                        # 0:6 of window t+1 (same psum pair for even t,
                        # previous pair's carry for odd t)
                        t0, t1_ = 2 * pi, 2 * pi + 1
                        nc.vector.tensor_add(
                            st[32:38, 256 * t0:256 * t0 + 256],
                            st[32:38, 256 * t0:256 * t0 + 256],
                            pq[0:6, 256:512])
                        if pi >= 1:
                            tp = 2 * pi - 1
                            nc.vector.tensor_add(
                                st[32:38, 256 * tp:256 * tp + 256],
                                st[32:38, 256 * tp:256 * tp + 256],
                                pq[0:6, 0:256])
                    # 3 DMAs -> bnci[c]
                    dstA = bnci[c][3:227, :].rearrange(
                        "(q r) x -> r q x", r=32)
                    srcA = st[6:38, 0:7 * 256].rearrange(
                        "r (q x) -> r q x", x=256)
                    nc.sync.dma_start(dstA, srcA)
                    nc.sync.dma_start(bnci[c][227:256, :],
                                      st[6:35, 7 * 256:8 * 256])
                    nc.sync.dma_start(bnci[c][0:3, :], st[3:6, 0:256])
                    nc.gpsimd.collective_compute(
                        "AllReduce", AX.add, replica_groups=RG,
                        ins=[bnci[c].opt()], outs=[bnco[c].opt()])

                # ---- load -r as soon as each AR lands
                mark(f'k{k}_loads')
                for c in range(C):
                    nc.sync.dma_start(
                        cblk(rn, c).rearrange("p (h x) -> p h x", x=256),
                        bnco[c][:].rearrange("(h p) x -> p h x", h=2))
                    nc.vector.tensor_sub(cblk(rbf, c), cblk(xcur, c),
                                         cblk(rn, c))

                # ---- window build (PE) + convT + a update
                for c in range(C):
                    mark(f'k{k}_convT{c}')
                    # extract the 8 windows of -r for this channel from rn
                    # via shift matmuls; scalar-copy psum -> rwin interior
                    rwv = rwin[:].rearrange("r (t x) -> r t x", x=X)
                    for pi in range(4):
                        pw = pwb.tile([WW, 512], F32, tag="wb")
                        for ti in range(2):
                            t = 2 * pi + ti
                            j, h = t % 4, t // 4
                            col = 256 * ti
                            nc.tensor.matmul(
                                pw[0:WW, col:col + 256], SH[j],
                                rbf[:, 512 * c + 256 * h:
                                    512 * c + 256 * h + 256],
                                start=True, stop=(t not in (3, 4)))
                            if t == 3:
                                nc.tensor.matmul(
                                    pw[0:WW, col:col + 256], SH[4],
                                    rbf[:, 512 * c + 256:512 * c + 512],
                                    start=False, stop=True)
                            if t == 4:
                                nc.tensor.matmul(
                                    pw[0:WW, col:col + 256], SH[5],
                                    rbf[:, 512 * c:512 * c + 256],
                                    start=False, stop=True)
                        dstP = rwv[:, NT * c + 2 * pi:NT * c + 2 * pi + 2,
                                   3:259]
                        nc.scalar.copy(
                            dstP, pw[:].rearrange("r (b x) -> r b x", x=256))
                    for g in range(2):
                        for cc in range(2):
                            tav = tpool.tile([128, 1024], F32, tag="tav")
                            for pp in range(2):
                                p2 = 2 * cc + pp
                                pd = pda.tile([128, 512], F32, tag="da")
                                mv = rwv[:, NT * c + 2 * p2:
                                         NT * c + 2 * p2 + 2, :]
                                for i, dx in enumerate(range(K7)):
                                    nc.tensor.matmul(
                                        pd[:], W2n[c, g, dx],
                                        mv[:, :, dx:dx + 256],
                                        start=(i == 0), stop=(i == K7 - 1))
                                av = A[c, g][:].rearrange(
                                    "p (t x) -> p t x", x=X)[
                                    :, 2 * p2:2 * p2 + 2, 3:259]
                                nc.vector.scalar_tensor_tensor(
                                    tav[:, 512 * pp:512 * pp + 512].rearrange(
                                        "p (b x) -> p b x", x=256),
                                    pd[:].rearrange("p (b x) -> p b x", x=256),
                                    scap(k, 1), av,
                                    op0=AX.mult, op1=AX.add)
                            s1 = tpool.tile([128, 1024], F32, tag="s1")
                            s2 = tpool.tile([128, 1024], F32, tag="s2")
                            nc.scalar.activation(s1[:], tav[:], AF.Relu,
                                                 bias=scap(k, 2), scale=1.0)
                            nc.scalar.activation(s2[:], tav[:], AF.Relu,
                                                 bias=scap(k, 2), scale=-1.0)
                            av4 = A[c, g][:].rearrange(
                                "p (t x) -> p t x", x=X)[
                                :, 4 * cc:4 * cc + 4, 3:259]
                            nc.vector.tensor_sub(
                                av4,
                                s1[:].rearrange("p (t x) -> p t x", x=256),
                                s2[:].rearrange("p (t x) -> p t x", x=256))

                # ---- deferred prox of the previous step (overlaps the
                # conv/convT PE work of this step)
                if k >= 1:
                    mark(f'k{k}_prox')
                    kp = k - 1
                    nc.scalar.square(tt1[:], y1a[:])
                    nc.scalar.square(tt2[:], y1b[:])
                    nc.vector.tensor_add(tt1[:], tt1[:], tt2[:])
                    nc.scalar.activation(tt2[:], tt1[:], AF.Ln,
                                         bias=scap(kp, 3), scale=1.0)
                    nc.scalar.activation(tt1[:], tt2[:], AF.Exp,
                                         bias=0.0, scale=-0.5)
                    nc.scalar.activation(tt2[:], tt1[:], AF.Relu,
                                         bias=1.0, scale=scap(kp, 5))
                    nc.scalar.activation(tt1[:], tt2[:], AF.Identity,
                                         bias=1.0, scale=scap(kp, 6))
                    nc.vector.tensor_mul(y1a[:], y1a[:], tt1[:])
                    nc.vector.tensor_mul(y1b[:], y1b[:], tt1[:])
                    nc.vector.scalar_tensor_tensor(
                        tt1[:], y2m[:], scap(kp, 7), zt[:],
                        op0=AX.mult, op1=AX.subtract)
                    nc.scalar.square(tt2[:], tt1[:])
                    nrs = tpool.tile([128, 1], F32, tag="nrs")
                    nc.vector.tensor_reduce(nrs[:], tt2[:], axis=AXL.X,
                                            op=AX.add)
                    tot = pbx.tile([128, 512], F32, tag="bx")
                    nc.tensor.matmul(tot[:, 0:1], ones128[:], nrs[:],
                                     start=True, stop=True)
                    fsc = tpool.tile([128, 1], F32, tag="fsc")
                    nc.scalar.sqrt(fsc[:], tot[:, 0:1])
                    nc.vector.tensor_scalar_max(fsc[:], fsc[:], float(EPS))
                    rc2 = tpool.tile([128, 1], F32, tag="rc2")
                    nc.vector.reciprocal(rc2[:], fsc[:])
                    nc.vector.tensor_scalar(fsc[:], rc2[:], EPS_R, 1.0,
                                            op0=AX.mult, op1=AX.min)
                    nc.vector.scalar_tensor_tensor(
                        tt2[:], tt1[:], fsc[:], zt[:],
                        op0=AX.mult, op1=AX.add)
                    for c in range(C):
                        nc.vector.scalar_tensor_tensor(
                            cblk(y2m, c), cblk(tt2, c), scap(kp, 8),
                            cblk(y2m, c), op0=AX.mult, op1=AX.add)

                # ---- dty = Dt(y1) + blur(y2) - r, then x update + v
                mark(f'k{k}_dty')
                nc.sync.dma_start(d0t[1:128, :], y1a[0:127, :])
                for h in range(2):
                    dsh = d0t[0:1, :].rearrange(
                        "p (c h x) -> p c h x", c=C, h=2)[:, :, h:h + 1, :]
                    ssh = y1a[127:128, :].rearrange(
                        "p (c h x) -> p c h x", c=C, h=2)[:, :, 1 - h:2 - h, :]
                    nc.sync.dma_start(dsh, ssh)
                nc.vector.tensor_sub(dty[:], d0t[:], y1a[:])
                v6o = tt1[:].rearrange("p (b x) -> p b x", x=256)
                v6i = y1b[:].rearrange("p (b x) -> p b x", x=256)
                nc.vector.tensor_sub(v6o[:, :, 1:256], v6i[:, :, 0:255],
                                     v6i[:, :, 1:256])
                nc.vector.tensor_sub(v6o[:, :, 0:1], v6i[:, :, 255:256],
                                     v6i[:, :, 0:1])
                nc.vector.tensor_add(dty[:], dty[:], tt1[:])

                for c in range(C):
                    def y2blur_consume(pxm, c=c):
                        nc.vector.tensor_add(cblk(dty, c), cblk(dty, c),
                                             pxm[:])

                    blur_c(y2m, c, y2blur_consume)
                    nc.vector.tensor_add(cblk(dty, c), cblk(dty, c),
                                          cblk(rbf, c))
                    nc.vector.scalar_tensor_tensor(
                        cblk(tt2, c), cblk(dty, c), scap(k, 0), cblk(xcur, c),
                        op0=AX.mult, op1=AX.add)
                    nc.vector.tensor_scalar(cblk(xnxt, c), cblk(tt2, c),
                                            0.0, 1.0, op0=AX.max, op1=AX.min)
                    nc.vector.scalar_tensor_tensor(
                        cblk(vm, c), cblk(xnxt, c), 2.0, cblk(xcur, c),
                        op0=AX.mult, op1=AX.subtract)

                    def y2_consume(pxm, c=c, k=k):
                        nc.vector.scalar_tensor_tensor(
                            cblk(y2m, c), pxm[:], scap(k, 4), cblk(y2m, c),
                            op0=AX.mult, op1=AX.add)

                    blur_c(vm, c, y2_consume)

                # ---- y1/y2 accumulate (prox deferred to next iteration)
                mark(f'k{k}_acc')
                nc.sync.dma_start(dvt[0:127, :], vm[1:128, :])
                for h in range(2):
                    dsh = dvt[127:128, :].rearrange(
                        "p (c h x) -> p c h x", c=C, h=2)[:, :, h:h + 1, :]
                    ssh = vm[0:1, :].rearrange(
                        "p (c h x) -> p c h x", c=C, h=2)[:, :, 1 - h:2 - h, :]
                    nc.sync.dma_start(dsh, ssh)
                nc.vector.tensor_sub(tt1[:], dvt[:], vm[:])
                nc.vector.scalar_tensor_tensor(
                    y1a[:], tt1[:], scap(k, 4), y1a[:],
                    op0=AX.mult, op1=AX.add)
                v6o = tt2[:].rearrange("p (b x) -> p b x", x=256)
                v6i = vm[:].rearrange("p (b x) -> p b x", x=256)
                nc.vector.tensor_sub(v6o[:, :, 0:255], v6i[:, :, 1:256],
                                     v6i[:, :, 0:255])
                nc.vector.tensor_sub(v6o[:, :, 255:256], v6i[:, :, 0:1],
                                     v6i[:, :, 255:256])
                nc.vector.scalar_tensor_tensor(
                    y1b[:], tt2[:], scap(k, 4), y1b[:],
                    op0=AX.mult, op1=AX.add)

                xcur, xnxt = xnxt, xcur

            # ---------------- output
            dstO = x_out[:].rearrange("c (h p) x -> p c h x", h=2)
            srcO = xcur[:].rearrange("p (c h x) -> p c h x", c=C, h=2)
            nc.sync.dma_start(dstO, srcO)

    _split_sync_waits(nc)
    return nc


# ---------------------------------------------------------------- host entry


def kernel(z, a_init, B, fil, lam1, lam2, gam1, gam2, gam3):
    from concourse.bass_utils import run_bass_kernel_spmd

    if "nc" not in _NC_CACHE:
        _NC_CACHE["nc"] = _build_nc()
    nc = _NC_CACHE["nc"]

    z = np.asarray(z, np.float32)
    a_init = np.asarray(a_init, np.float32)
    B = np.asarray(B, np.float32)
    fil = np.asarray(fil, np.float32)
    sc = _build_sc(np.asarray(lam1, np.float32), np.asarray(lam2, np.float32),
                   np.asarray(gam1, np.float32), np.asarray(gam2, np.float32),
                   np.asarray(gam3, np.float32))
    Wy, Cu, Cd, Dg = _build_sep_blur(fil)
    Shw = _build_shift_w()
    on1 = np.ones((1, 128), np.float32)
    on128 = np.ones((128, 128), np.float32)

    in_maps = []
    for core in range(8):
        b, pg = core // 4, core % 4
        Bsh = B[PL * pg:PL * pg + PL]
        in_maps.append({
            "a_in": np.ascontiguousarray(
                a_init[b, PL * pg:PL * pg + PL]).astype(BF),
            "zr_in": np.ascontiguousarray(z[b]).astype(BF),
            "zf_in": np.ascontiguousarray(z[b]),
            "w1_in": _build_w1(Bsh),
            "w2_in": _build_w2n(Bsh),
            "wy_in": Wy, "cu_in": Cu, "cd_in": Cd, "dg_in": Dg,
            "shw_in": Shw,
            "sc_in": sc, "on1_in": on1, "on128_in": on128,
        })
    global _LAST_IN_MAPS
    _LAST_IN_MAPS = in_maps
    res = run_bass_kernel_spmd(nc, in_maps, core_ids=list(range(8)))
    out = np.stack([res.results[0]["x_out"], res.results[4]["x_out"]])
    return out.astype(np.float32)
